# revision 1
# baseline (speedup 1.0000x reference)
"""GNN message-passing kernel for TRN2, one batch element per NeuronCore.

Per-core math (x: [W=2048, C=512], weights replicated):
  cw    = sigmoid(relu(mean_W(x)@avg_w.T) + relu(max_W(x)@max_w.T))   [M=128]
  xxT   = fc_w @ xT + b                         [M, W]     (f32r matmuls)
  dqT   = cw * xxT
  S     = dqT.T @ xxT  is symmetric -> compute only the upper trapezoid per
          row block; deg = trapezoid row sums (ACT sigmoid accum_out) plus
          column sums of off-diagonal tiles (PE ones-matmuls on sigmoid(S),
          accumulated across row blocks in one persistent PSUM bank).
  d     = deg^-1/2                              (DVE Newton rsqrt, 3 iters)
  G1    = x @ gcn_w     (lhsT = xT slices)      [W, C] f32r
  P     = d * xx        (in-place scale of PE-transposed xxT blocks); PT = P^T
  T1    = P^T @ G1 ;  T2 = (-cw) * T1           [M, C]
  out_i = PT_i.T @ T2 + I.T @ G1_i  (accumulating matmuls / fused DVE add)
"""

from contextlib import ExitStack

import numpy as np

import concourse.bass as bass
import concourse.tile as tile
from concourse import bacc, mybir

f32 = mybir.dt.float32
f32r = mybir.dt.float32r
f16 = mybir.dt.float16
AF = mybir.ActivationFunctionType
ALU = mybir.AluOpType

W, C, M = 2048, 512, 128
CQ = C // 128      # 4 c-chunks
NH = 2 * CQ        # 8 half-chunks for stats
NW = W // 128      # 16 w-chunks
WS = W // 512      # 4 w-slices


def round_fp32r(a: np.ndarray) -> np.ndarray:
    bits = np.ascontiguousarray(a).view(np.uint32)
    r = ((bits.astype(np.uint64) + 0x800) & 0xFFFFF000).astype(np.uint32)
    return r.view(np.float32)


def build_nc():
    nc = bacc.Bacc("TRN2", target_bir_lowering=False, debug=False, num_devices=8)

    xT_d = nc.dram_tensor("xT", [C, W], f32r, kind="ExternalInput").ap()
    fcwT_d = nc.dram_tensor("fcwT", [C, M], f32r, kind="ExternalInput").ap()
    avgwT_d = nc.dram_tensor("avgwT", [C, M], f32, kind="ExternalInput").ap()
    maxwT_d = nc.dram_tensor("maxwT", [C, M], f32, kind="ExternalInput").ap()
    fcb_d = nc.dram_tensor("fcb", [M, 1], f32, kind="ExternalInput").ap()
    gcn_d = nc.dram_tensor("gcn", [C, C], f32r, kind="ExternalInput").ap()
    ident_d = nc.dram_tensor("ident", [128, 128], f32r, kind="ExternalInput").ap()
    out_d = nc.dram_tensor("out", [W, C], f32, kind="ExternalOutput").ap()

    with tile.TileContext(nc) as tc, ExitStack() as ctx:
        pool = ctx.enter_context(tc.tile_pool(name="sb", bufs=1))
        sigp = ctx.enter_context(tc.tile_pool(name="sigp", bufs=4))
        outp = ctx.enter_context(tc.tile_pool(name="outp", bufs=3))
        psS = ctx.enter_context(tc.tile_pool(name="psS", bufs=2, space="PSUM"))
        psA = ctx.enter_context(tc.tile_pool(name="psA", bufs=2, space="PSUM"))
        psB = ctx.enter_context(tc.tile_pool(name="psB", bufs=1, space="PSUM"))
        psC = ctx.enter_context(tc.tile_pool(name="psC", bufs=1, space="PSUM"))

        # ---------- persistent SBUF tensors ----------
        xT = pool.tile([128, CQ, W], f32r)          # x^T, c-chunk k on partitions
        fcwT = pool.tile([128, CQ, M], f32r)
        avgwT = pool.tile([128, CQ, M], f32)
        maxwT = pool.tile([128, CQ, M], f32)
        fcb = pool.tile([128, 1], f32)
        gcn = pool.tile([128, CQ, C], f32r)
        ident = pool.tile([128, 128], f32r)
        xxT = pool.tile([128, W], f32r)             # fc_w @ x^T + b   [M, W]
        dqT = pool.tile([128, W], f32r)             # cw * xxT
        xx_nat = pool.tile([128, NW, 128], f32r)    # xx w-chunks; scaled in place by d -> P
        PT = pool.tile([128, W], f32r)              # P^T  [M, W]
        G1 = pool.tile([128, NW, C], f32r)          # x @ gcn_w, w-chunk i on partitions
        T2 = pool.tile([128, C], f32r)
        dump = pool.tile([128, W // 2], f32)        # ACT stats dump target
        xsum = pool.tile([128, NH], f32)
        xmax = pool.tile([128, NH], f32)
        xmax2 = pool.tile([128, CQ], f32)
        a_sb = pool.tile([128, 1], f32)
        m_sb = pool.tile([128, 1], f32)
        cw = pool.tile([128, 1], f32)
        ncw = pool.tile([128, 1], f32)
        ones = pool.tile([128, 1], f32)
        zeros1 = pool.tile([128, 1], f32)
        scr1 = pool.tile([128, 1], f32)
        deg_parts = pool.tile([128, NW, 2], f32)
        acc_cs = pool.tile([128, NW], f32)          # accumulated column sums (SBUF)
        deg = pool.tile([128, NW], f32)
        y_nr = pool.tile([128, NW], f32)            # rsqrt iterate -> d
        t_nr = pool.tile([128, NW], f32)
        u_nr = pool.tile([128, NW], f32)

        # Pin the ACT table set: make the first ACT instruction a Sigmoid.
        nc.gpsimd.memset(zeros1[:], 0.0)
        nc.scalar.activation(scr1[:], zeros1[:], AF.Sigmoid)
        nc.vector.memset(y_nr[:], 1.0 / 32.0)
        nc.vector.memset(ones[:], 1.0)
        nc.vector.memset(deg_parts[:].rearrange("p a b -> p (a b)"), 0.0)
        nc.vector.memset(acc_cs[:], 0.0)

        # ---------- loads: small weights, then xT half-chunks, gcn last ----------
        nc.sync.dma_start(fcwT[:], fcwT_d.rearrange("(k p) m -> p k m", p=128))
        nc.sync.dma_start(avgwT[:], avgwT_d.rearrange("(k p) m -> p k m", p=128))
        nc.sync.dma_start(maxwT[:], maxwT_d.rearrange("(k p) m -> p k m", p=128))
        nc.sync.dma_start(fcb[:], fcb_d[:])
        nc.sync.dma_start(ident[:], ident_d[:])
        for h in range(NH):
            k, p = h // 2, h % 2
            nc.sync.dma_start(xT[:, k, bass.ts(p, W // 2)], xT_d[bass.ts(k, 128), bass.ts(p, W // 2)])
        nc.sync.dma_start(gcn[:], gcn_d.rearrange("(k p) c -> p k c", p=128))

        # ---------- stats per half-chunk: max (DVE); sum split ACT/DVE ----------
        for h in range(NH):
            k, p = h // 2, h % 2
            sl = xT[:, k, bass.ts(p, W // 2)].bitcast(f32)
            nc.vector.reduce_max(xmax[:, h : h + 1], sl, axis=mybir.AxisListType.X)
            nc.scalar.activation(dump[:], sl, AF.Copy, accum_out=xsum[:, h : h + 1])


        def cw_mms():
            pa = psB.tile([128, 512], f32, tag="b")
            for h in range(NH):
                nc.tensor.matmul(pa[:, 0:1], avgwT[:, h // 2, :], xsum[:, h : h + 1], start=(h == 0), stop=(h == NH - 1))
            nc.scalar.activation(a_sb[:], pa[:, 0:1], AF.Relu, scale=1.0 / W)
            nc.vector.reduce_max(xmax2[:], xmax[:].rearrange("p (k h) -> p k h", h=2), axis=mybir.AxisListType.X)
            pm = psB.tile([128, 512], f32, tag="b")
            for k in range(CQ):
                nc.tensor.matmul(pm[:, 0:1], maxwT[:, k, :], xmax2[:, k : k + 1], start=(k == 0), stop=(k == CQ - 1))
            nc.scalar.activation(m_sb[:], pm[:, 0:1], AF.Relu)
            nc.scalar.activation(cw[:], a_sb[:], AF.Sigmoid, bias=m_sb[:, 0:1])
            nc.vector.tensor_scalar_mul(ncw[:], cw[:], -1.0)

        def g1_tile(i, copy_eng):
            pg = psA.tile([128, 512], f32, tag="a")
            for k in range(CQ):
                nc.tensor.matmul(pg[:], xT[:, k, bass.ts(i, 128)], gcn[:, k, :], start=(k == 0), stop=(k == CQ - 1))
            if copy_eng == 0:
                nc.vector.tensor_copy(G1[:, i, :], pg[:])
            else:
                nc.scalar.activation(G1[:, i, :], pg[:], AF.Copy)

        # ---------- xxT = fc_w @ xT + b  (cw matmuls slotted between slices) ----------
        for s in range(WS):
            px = psA.tile([128, 512], f32, tag="a")
            for k in range(CQ):
                nc.tensor.matmul(px[:], fcwT[:, k, :], xT[:, k, bass.ts(s, 512)], start=(k == 0), stop=(k == CQ - 1))
            nc.vector.tensor_scalar_add(xxT[:, bass.ts(s, 512)], px[:], fcb[:, 0:1])
            if s == 1:
                cw_mms()

        # dqT = cw * xxT (split so S block 0 can start early)
        for s in range(WS):
            nc.vector.tensor_scalar_mul(dqT[:, bass.ts(s, 512)], xxT[:, bass.ts(s, 512)].bitcast(f32), cw[:, 0:1])

        pt1 = None
        cs = None

        def d_chain(lo, hi, iters=3):
            """rsqrt(deg) for chunks [lo, hi) on DVE; scale xx_nat -> P in place."""
            sl = slice(lo, hi)
            nc.vector.reduce_sum(deg[:, sl], deg_parts[:, sl, :], axis=mybir.AxisListType.X)
            csl = slice(max(lo, 1), hi)  # column chunk 0 receives no colsums
            nc.vector.tensor_tensor(deg[:, csl], deg[:, csl], acc_cs[:, csl], op=ALU.add)
            for _ in range(iters):
                nc.vector.tensor_tensor(t_nr[:, sl], y_nr[:, sl], y_nr[:, sl], op=ALU.mult)
                nc.vector.tensor_tensor(t_nr[:, sl], t_nr[:, sl], deg[:, sl], op=ALU.mult)
                nc.vector.tensor_scalar(u_nr[:, sl], t_nr[:, sl], -0.5, 1.5, op0=ALU.mult, op1=ALU.add)
                nc.vector.tensor_tensor(y_nr[:, sl], y_nr[:, sl], u_nr[:, sl], op=ALU.mult)
            for i in range(lo, hi):
                nc.vector.tensor_scalar_mul(xx_nat[:, i, :], xx_nat[:, i, :].bitcast(f32), y_nr[:, i : i + 1])

        def t1_mms(lo, hi):
            nonlocal pt1
            if pt1 is None:
                pt1 = psB.tile([128, 512], f32, tag="b")
            for i in range(lo, hi):
                nc.tensor.matmul(pt1[:], xx_nat[:, i, :], G1[:, i, :], start=(i == 0), stop=(i == NW - 1))

        def pt_trans(lo, hi, copy_eng=0):
            ptp = psA.tile([128, 512], f32, tag="a")
            for i in range(lo, hi):
                nc.tensor.transpose(
                    ptp[:, bass.ts(i - lo, 128)].bitcast(f32r), xx_nat[:, i, :], ident[:]
                )
            w = 128 * (hi - lo)
            if copy_eng == 0:
                nc.vector.tensor_copy(PT[:, 128 * lo : 128 * lo + w], ptp[:, 0:w])
            else:
                nc.scalar.activation(PT[:, 128 * lo : 128 * lo + w], ptp[:, 0:w], AF.Copy)

        # G1 pacing: 1 tile per iter for i<4, then 2 per iter for i in 4..9
        g1_sched = {i: [i] for i in range(4)}
        nxt = 4
        for i in range(4, 10):
            g1_sched[i] = [nxt, nxt + 1]
            nxt += 2

        # ---------- S phase: upper trapezoid + column sums ----------
        for i in range(NW):
            start_col = 128 * i
            parts = []
            c0 = start_col
            while c0 < W:
                w = min(1024, W - c0)
                parts.append((c0, w))
                c0 += w
            sig_tiles = []
            for pidx, (c0, w) in enumerate(parts):
                ps = psS.tile([128, 1024], f32, tag="s")
                o = 0
                while o < w:
                    n = min(512, w - o)
                    nc.tensor.matmul(
                        ps[:, o : o + n],
                        dqT[:, bass.ts(i, 128)],
                        xxT[:, c0 + o : c0 + o + n],
                        start=True,
                        stop=True,
                    )
                    o += n
                sg = sigp.tile([128, 1024], f32, tag="sg")
                nc.scalar.activation(
                    sg[:, 0:w], ps[:, 0:w], AF.Sigmoid, accum_out=deg_parts[:, i, pidx : pidx + 1]
                )
                sig_tiles.append((sg, c0, w))
            # column sums of off-diagonal chunks; per-block PSUM group + DVE accumulate
            if i < NW - 1:
                cs = psC.tile([128, NW], f32, tag="c")
                first = True
                for sg, c0, w in sig_tiles:
                    j0 = max(c0 // 128, i + 1)
                    for j in range(j0, (c0 + w) // 128):
                        nc.tensor.matmul(
                            cs[:, j : j + 1],
                            sg[:, 128 * j - c0 : 128 * (j + 1) - c0],
                            ones[:],
                            start=first,
                            stop=(j == NW - 1),
                        )
                        first = False
                nc.vector.tensor_tensor(
                    acc_cs[:, i + 1 : NW], acc_cs[:, i + 1 : NW], cs[:, i + 1 : NW], op=ALU.add
                )
            for gi in g1_sched.get(i, []):
                g1_tile(gi, 0)
            # xx_nat fill: transpose 4 blocks of xxT per group (group 3 early, at iter 13)
            if i in (3, 7, 11, 13):
                g = 3 if i == 13 else i // 4
                pt = psA.tile([128, 512], f32, tag="a")
                for q in range(4):
                    blk = 4 * g + q
                    nc.tensor.transpose(
                        pt[:, bass.ts(q, 128)].bitcast(f32r), xxT[:, bass.ts(blk, 128)], ident[:]
                    )
                nc.vector.tensor_copy(xx_nat[:, 4 * g : 4 * (g + 1), :].rearrange("p a b -> p (a b)"), pt[:])
            if i in (4, 8, 12):
                g = (i - 4) // 4
                d_chain(4 * g, 4 * g + 4)
            if i in (5, 9, 13):
                g = (i - 5) // 4
                t1_mms(4 * g, 4 * g + 4)
                pt_trans(4 * g, 4 * g + 4)

        d_chain(12, 16, iters=2)
        t1_mms(12, 16)
        pt_trans(12, 16, copy_eng=1)

        # T2 = (-cw) * T1 on DVE (runs parallel to the ACT PT copy)
        nc.vector.tensor_scalar_mul(T2[:], pt1[:], ncw[:, 0:1])

        # ---------- out_i = G1_i - d_i*(xx @ cw*T1)_i; store per 2 pairs, DVE pair first ----------
        for g in range(4):
            st = outp.tile([128, 4, 512], f32)
            for p in (2 * g + 1, 2 * g):
                pq = p - 2 * g
                py2 = psS.tile([128, 1024], f32, tag="s")
                dst = st[:, 2 * pq : 2 * pq + 2, :].rearrange("p a b -> p (a b)")
                if p % 2 == 0:
                    for q in range(2):
                        i = 2 * p + q
                        nc.tensor.matmul(py2[:, bass.ts(q, 512)], PT[:, bass.ts(i, 128)], T2[:], start=True, stop=False)
                        nc.tensor.matmul(py2[:, bass.ts(q, 512)], ident[:], G1[:, i, :], start=False, stop=True)
                    nc.scalar.activation(dst, py2[:], AF.Copy)
                else:
                    for q in range(2):
                        i = 2 * p + q
                        nc.tensor.matmul(py2[:, bass.ts(q, 512)], PT[:, bass.ts(i, 128)], T2[:], start=True, stop=True)
                    g1p = G1[:, 2 * p : 2 * p + 2, :].rearrange("p a b -> p (a b)").bitcast(f32)
                    nc.vector.tensor_tensor(dst, py2[:], g1p, op=ALU.add)
            if g < 3:
                nc.sync.dma_start(out_d[bass.ts(g, 512), :].rearrange("(q p) c -> p q c", p=128), st[:])
            else:
                for pq in range(2):
                    nc.sync.dma_start(
                        out_d[1536 + 256 * pq : 1792 + 256 * pq, :].rearrange("(q p) c -> p q c", p=128),
                        st[:, 2 * pq : 2 * pq + 2, :],
                    )

    nc.compile()
    return nc


def make_in_map(x_b, fc_w, fc_b, avg_w, max_w, gcn_w):
    """Per-core input map from this core's batch slice (all np.float32)."""
    return {
        "xT": round_fp32r(np.ascontiguousarray(x_b.T)),
        "fcwT": round_fp32r(np.ascontiguousarray(fc_w.T)),
        "avgwT": np.ascontiguousarray(avg_w.T),
        "maxwT": np.ascontiguousarray(max_w.T),
        "fcb": np.ascontiguousarray(fc_b.reshape(M, 1)),
        "gcn": round_fp32r(np.ascontiguousarray(gcn_w)),
        "ident": np.eye(128, dtype=np.float32),
    }


# ======================================================================
# Harness entry point: full inputs in, full output out.
# Shards batch B=8 across the 8 NeuronCores (pure data parallel) and runs
# the Bass kernel via run_bass_kernel_spmd.
# ======================================================================

_NC_CACHE = None


def _get_nc():
    global _NC_CACHE
    if _NC_CACHE is None:
        _NC_CACHE = build_nc()
    return _NC_CACHE


def make_in_maps(x, fc_w, fc_b, avg_w, max_w, gcn_w):
    x = np.asarray(x, dtype=np.float32)
    fc_w = np.asarray(fc_w, dtype=np.float32)
    fc_b = np.asarray(fc_b, dtype=np.float32)
    avg_w = np.asarray(avg_w, dtype=np.float32)
    max_w = np.asarray(max_w, dtype=np.float32)
    gcn_w = np.asarray(gcn_w, dtype=np.float32)
    shared = {
        "fcwT": round_fp32r(np.ascontiguousarray(fc_w.T)),
        "avgwT": np.ascontiguousarray(avg_w.T),
        "maxwT": np.ascontiguousarray(max_w.T),
        "fcb": np.ascontiguousarray(fc_b.reshape(M, 1)),
        "gcn": round_fp32r(np.ascontiguousarray(gcn_w)),
        "ident": np.eye(128, dtype=np.float32),
    }
    return [
        {"xT": round_fp32r(np.ascontiguousarray(x[b].T)), **shared}
        for b in range(x.shape[0])
    ]


def kernel(x, fc_w, fc_b, avg_w, max_w, gcn_w):
    from concourse.bass_utils import run_bass_kernel_spmd

    nc = _get_nc()
    in_maps = make_in_maps(x, fc_w, fc_b, avg_w, max_w, gcn_w)
    res = run_bass_kernel_spmd(nc, in_maps, list(range(len(in_maps))))
    out = np.stack([res.results[b]["out"] for b in range(len(in_maps))])
    return out.astype(np.float32)



# revision 37
# speedup vs baseline: 1.2627x; 1.2627x over previous
"""GNN message-passing kernel for TRN2, one batch element per NeuronCore.

Per-core math (x: [W=2048, C=512], weights replicated), all-bf16 dataflow:
  cw    = sigmoid(relu(mean_W(x)@avg_w.T) + relu(max_W(x)@max_w.T))   [M=128]
  xxT   = fc_w @ xT + b                         [M, W]
  dqT   = cw * xxT
  S     = dqT.T @ xxT  is symmetric -> upper trapezoid per row block;
          deg = trapezoid row sums (ACT sigmoid accum_out) + column sums of
          off-diagonal tiles (PE ones-matmuls on sigmoid(S) in bf16).
  d     = deg^-1/2   (DVE Newton rsqrt)
  G1    = x @ gcn_w                             [W, C]
  P     = d * xx  (nat layout, via PE transposes); PT = P^T
  T1    = P^T @ G1 ;  T2 = (-cw) * T1           [M, C]
  out_i = G1_i - PT_i.T @ (cw*T1)   (PE matmuls; G1-add fused into the
          PSUM->SBUF copy on DVE, or via PE ident-matmul for ACT copies)

Inputs are uploaded as bf16 (halves DMA), output is stored as bf16 and
converted to f32 on the host. Chunks 12-15 of the d/P/T1 pipeline run
per-chunk right behind the last S row blocks to shrink the tail.
"""

from contextlib import ExitStack

import numpy as np

import concourse.bass as bass
import concourse.tile as tile
from concourse import bacc, mybir

f32 = mybir.dt.float32
bf16 = mybir.dt.bfloat16
AF = mybir.ActivationFunctionType
ALU = mybir.AluOpType

W, C, M = 2048, 512, 128
CQ = C // 128      # 4 c-chunks
NW = W // 128      # 16 w-chunks
WS = W // 512      # 4 w-slices


def build_nc():
    nc = bacc.Bacc("TRN2", target_bir_lowering=False, debug=False, num_devices=8)

    xT_d = nc.dram_tensor("xT", [C, W], bf16, kind="ExternalInput").ap()
    fcwT_d = nc.dram_tensor("fcwT", [128, CQ * M], bf16, kind="ExternalInput").ap()
    avgwT_d = nc.dram_tensor("avgwT", [128, CQ * M], bf16, kind="ExternalInput").ap()
    maxwT_d = nc.dram_tensor("maxwT", [128, CQ * M], bf16, kind="ExternalInput").ap()
    fcb_d = nc.dram_tensor("fcb", [1, M], bf16, kind="ExternalInput").ap()
    gcn_d = nc.dram_tensor("gcn", [128, CQ * C], bf16, kind="ExternalInput").ap()
    ident_d = nc.dram_tensor("ident", [128, 128], bf16, kind="ExternalInput").ap()
    out_d = nc.dram_tensor("out", [W, C], bf16, kind="ExternalOutput").ap()

    with tile.TileContext(nc) as tc, ExitStack() as ctx:
        pool = ctx.enter_context(tc.tile_pool(name="sb", bufs=1))
        sigp = ctx.enter_context(tc.tile_pool(name="sigp", bufs=6))
        outp = ctx.enter_context(tc.tile_pool(name="outp", bufs=8))
        psS = ctx.enter_context(tc.tile_pool(name="psS", bufs=2, space="PSUM"))
        psA = ctx.enter_context(tc.tile_pool(name="psA", bufs=2, space="PSUM"))
        psB = ctx.enter_context(tc.tile_pool(name="psB", bufs=1, space="PSUM"))
        psC = ctx.enter_context(tc.tile_pool(name="psC", bufs=1, space="PSUM"))

        # ---------- persistent SBUF tensors ----------
        xT = pool.tile([128, CQ, W], bf16)          # x^T, c-chunk k on partitions
        fcwT = pool.tile([128, CQ, M], bf16)
        avgwT = pool.tile([128, CQ, M], bf16)
        maxwT = pool.tile([128, CQ, M], bf16)
        gcn = pool.tile([128, CQ, C], bf16)
        ident = pool.tile([128, 128], bf16)
        xxT = pool.tile([128, W], bf16)             # fc_w @ x^T + b   [M, W]
        dqT = pool.tile([128, W], bf16)             # cw * xxT
        xx_nat = pool.tile([128, NW, 128], bf16)    # xx w-chunks; scaled in place by d -> P
        PT = pool.tile([128, W], bf16)              # P^T  [M, W]
        G1 = pool.tile([128, NW, C], bf16)          # x @ gcn_w, w-chunk i on partitions
        T2 = pool.tile([128, C], bf16)
        dump = pool.tile([128, W], bf16)            # ACT stats dump target
        mt1 = pool.tile([128, W // 2], bf16)        # max-tree scratch
        mt2 = pool.tile([128, W // 4], bf16)
        xsum_f = pool.tile([128, 8], f32)
        xmax_f = pool.tile([128, 4], f32)
        xsum_b = pool.tile([128, 4], bf16)
        xmax_b = pool.tile([128, 4], bf16)
        a_sb = pool.tile([128, 1], f32)
        m_sb = pool.tile([128, 1], f32)
        cw = pool.tile([128, 1], f32)
        ncw = pool.tile([128, 1], f32)
        ones = pool.tile([128, 1], bf16)
        ones_row = pool.tile([1, 512], bf16)
        fcb_row = pool.tile([1, 128], bf16)
        zeros1 = pool.tile([128, 1], f32)
        scr1 = pool.tile([128, 1], f32)
        deg_parts = pool.tile([128, NW, 2], f32)
        acc_cs = pool.tile([128, NW], f32)          # accumulated column sums (SBUF)
        deg = pool.tile([128, NW], f32)
        y_nr = pool.tile([128, NW], f32)            # rsqrt iterate -> d
        t_nr = pool.tile([128, NW], f32)
        u_nr = pool.tile([128, NW], f32)

        # Pin the ACT table set: make the first ACT instruction a Sigmoid.
        nc.gpsimd.memset(zeros1[:], 0.0)
        nc.scalar.activation(scr1[:], zeros1[:], AF.Sigmoid)
        nc.vector.memset(y_nr[:], 1.0 / 32.0)
        nc.vector.memset(ones[:], 1.0)
        nc.vector.memset(ones_row[:], 1.0)
        nc.vector.memset(deg_parts[:].rearrange("p a b -> p (a b)"), 0.0)
        nc.vector.memset(acc_cs[:], 0.0)

        # ---------- loads: fcwT, xT halves (k-major), small weights, gcn ----------
        nc.sync.dma_start(fcwT[:].rearrange("p k m -> p (k m)"), fcwT_d[:])
        for h in range(2 * CQ):
            k, p = h // 2, h % 2
            nc.sync.dma_start(xT[:, k, bass.ts(p, W // 2)], xT_d[bass.ts(k, 128), bass.ts(p, W // 2)])
        nc.sync.dma_start(avgwT[:].rearrange("p k m -> p (k m)"), avgwT_d[:])
        nc.sync.dma_start(maxwT[:].rearrange("p k m -> p (k m)"), maxwT_d[:])
        nc.sync.dma_start(fcb_row[:], fcb_d[:])
        nc.sync.dma_start(ident[:], ident_d[:])
        nc.sync.dma_start(gcn[:].rearrange("p k c -> p (k c)"), gcn_d[:])

        # ---------- stats during load ----------
        # ACT: full-chunk sums for chunks 0..2, then second half of chunk 3
        for k in range(3):
            nc.scalar.activation(dump[:], xT[:, k, :], AF.Copy, accum_out=xsum_f[:, k : k + 1])
        nc.scalar.activation(dump[:, 0 : W // 2], xT[:, 3, W // 2 : W], AF.Copy, accum_out=xsum_f[:, 5:6])

        # DVE: max trees (Pool/gpsimd cannot run generic elementwise ops on
        # real hardware, so everything lands on DVE), chunk-3 first-half sum
        def max_tree(k):
            nc.vector.tensor_tensor(mt1[:], xT[:, k, 0 : W // 2], xT[:, k, W // 2 : W], op=ALU.max)
            nc.vector.tensor_tensor(mt2[:], mt1[:, 0 : W // 4], mt1[:, W // 4 : W // 2], op=ALU.max)
            nc.vector.tensor_tensor(mt1[:, 0 : W // 8], mt2[:, 0 : W // 8], mt2[:, W // 8 : W // 4], op=ALU.max)
            nc.vector.reduce_max(xmax_f[:, k : k + 1], mt1[:, 0 : W // 8], axis=mybir.AxisListType.X)

        for k in range(3):
            max_tree(k)
        nc.vector.reduce_sum(xsum_f[:, 4:5], xT[:, 3, 0 : W // 2], axis=mybir.AxisListType.X)
        max_tree(3)

        # ---------- xxT = fc_w @ xT + b: bias folded in as a rank-1 matmul
        # (fcb_row x ones_row) so the PSUM->SBUF copies are plain Copies,
        # split ACT (s0,s1) / DVE (s2,s3). gpsimd cannot read PSUM. ----------
        px0 = psA.tile([128, 512], f32, tag="a")
        px1 = psA.tile([128, 512], f32, tag="a")
        px23 = psS.tile([128, 1024], f32, tag="s")
        pxs = [px0[:], px1[:], px23[:, 0:512], px23[:, 512:1024]]
        for k in range(CQ):
            for s in (0, 1):
                nc.tensor.matmul(pxs[s], fcwT[:, k, :], xT[:, k, bass.ts(s, 512)], start=(k == 0), stop=False)
        for s in (0, 1):
            nc.tensor.matmul(pxs[s], fcb_row[:], ones_row[:], start=False, stop=True)
        for k in range(CQ):
            for s in (2, 3):
                nc.tensor.matmul(pxs[s], fcwT[:, k, :], xT[:, k, bass.ts(s, 512)], start=(k == 0), stop=False)
        for s in (2, 3):
            nc.tensor.matmul(pxs[s], fcb_row[:], ones_row[:], start=False, stop=True)
        # ---------- cw (emitted before the xxT copies so the relu/sigmoid
        # chain outranks them in ACT priority) ----------
        for k in range(3):
            nc.vector.tensor_copy(xsum_b[:, k : k + 1], xsum_f[:, k : k + 1])
        nc.vector.tensor_tensor(xsum_b[:, 3:4], xsum_f[:, 4:5], xsum_f[:, 5:6], op=ALU.add)
        nc.vector.tensor_copy(xmax_b[:], xmax_f[:])
        pa = psC.tile([128, 16], f32, tag="c")
        for k in range(CQ):
            nc.tensor.matmul(pa[:, 0:1], avgwT[:, k, :], xsum_b[:, k : k + 1], start=(k == 0), stop=(k == CQ - 1))
        nc.scalar.activation(a_sb[:], pa[:, 0:1], AF.Relu, scale=1.0 / W)
        pm = psC.tile([128, 16], f32, tag="c")
        for k in range(CQ):
            nc.tensor.matmul(pm[:, 0:1], maxwT[:, k, :], xmax_b[:, k : k + 1], start=(k == 0), stop=(k == CQ - 1))
        nc.scalar.activation(m_sb[:], pm[:, 0:1], AF.Relu)
        nc.scalar.activation(cw[:], a_sb[:], AF.Sigmoid, bias=m_sb[:, 0:1])
        nc.vector.tensor_scalar_mul(ncw[:], cw[:], -1.0)

        for s in (0, 1):
            nc.scalar.activation(xxT[:, bass.ts(s, 512)], pxs[s], AF.Copy)
        for s in (2, 3):
            nc.vector.tensor_copy(xxT[:, bass.ts(s, 512)], pxs[s])

        # dqT = cw * xxT (bf16 4x TSP)
        for s in range(WS):
            nc.vector.tensor_scalar_mul(dqT[:, bass.ts(s, 512)], xxT[:, bass.ts(s, 512)], cw[:, 0:1])



        pt1 = None

        def g1_tile(i):
            pg = psA.tile([128, 512], f32, tag="a")
            for k in range(CQ):
                nc.tensor.matmul(pg[:], xT[:, k, bass.ts(i, 128)], gcn[:, k, :], start=(k == 0), stop=(k == CQ - 1))
            nc.vector.tensor_copy(G1[:, i, :], pg[:])

        def xpose_group(g):
            """Transpose 4 xxT blocks of group g into xx_nat (bf16 PSUM path).

            The PSUM->SBUF copy runs on Pool: keeping it off DVE avoids
            stealing DVE slots from the stats chain that gates cw."""
            pt = psA.tile([128, 512], bf16, tag="a")
            for q in range(4):
                blk = 4 * g + q
                nc.tensor.transpose(pt[:, bass.ts(q, 128)], xxT[:, bass.ts(blk, 128)], ident[:])
            nc.vector.tensor_copy(xx_nat[:, 4 * g : 4 * (g + 1), :].rearrange("p a b -> p (a b)"), pt[:])

        def d_chain(lo, hi, iters=3, eng=None):
            """rsqrt(deg) for chunks [lo, hi); scale xx_nat -> P in place.

            eng=gpsimd runs the SBUF-only math on the idle Pool engine
            (mid-phase groups); the latency-critical tail chunks use DVE.
            First Newton step from the constant seed y0=1/32 collapses to
            one affine op: y1 = y0*(1.5 - 0.5*deg*y0^2) = 3/64 - deg/65536."""
            e = eng if eng is not None else nc.vector
            sl = slice(lo, hi)
            e.tensor_tensor(deg[:, sl], deg_parts[:, sl, 0:1].rearrange("p a b -> p (a b)"),
                            deg_parts[:, sl, 1:2].rearrange("p a b -> p (a b)"), op=ALU.add)
            csl = slice(max(lo, 1), hi)  # column chunk 0 receives no colsums
            nc.vector.tensor_tensor(deg[:, csl], deg[:, csl], acc_cs[:, csl], op=ALU.add)
            e.tensor_scalar(y_nr[:, sl], deg[:, sl], -1.0 / 65536.0, 3.0 / 64.0, op0=ALU.mult, op1=ALU.add)
            for _ in range(iters - 1):
                e.tensor_tensor(t_nr[:, sl], y_nr[:, sl], y_nr[:, sl], op=ALU.mult)
                e.tensor_tensor(t_nr[:, sl], t_nr[:, sl], deg[:, sl], op=ALU.mult)
                e.tensor_scalar(u_nr[:, sl], t_nr[:, sl], -0.5, 1.5, op0=ALU.mult, op1=ALU.add)
                e.tensor_tensor(y_nr[:, sl], y_nr[:, sl], u_nr[:, sl], op=ALU.mult)
            for i in range(lo, hi):
                e.tensor_scalar_mul(xx_nat[:, i, :], xx_nat[:, i, :], y_nr[:, i : i + 1])

        def t1_mms(lo, hi):
            nonlocal pt1
            if pt1 is None:
                pt1 = psB.tile([128, 512], f32, tag="b")
            for i in range(lo, hi):
                nc.tensor.matmul(pt1[:], xx_nat[:, i, :], G1[:, i, :], start=(i == 0), stop=(i == NW - 1))

        def pt_trans(lo, hi):
            ptp = psA.tile([128, 512], bf16, tag="a")
            for i in range(lo, hi):
                nc.tensor.transpose(ptp[:, bass.ts(i - lo, 128)], xx_nat[:, i, :], ident[:])
            w = 128 * (hi - lo)
            nc.vector.tensor_copy(PT[:, 128 * lo : 128 * lo + w], ptp[:, 0:w])

        def tail_chunk(c, iters=1):
            """Per-chunk d -> scale -> T1 matmul -> PT transpose (chunks 12-15).

            One Newton iteration suffices here: the 1/32 seed is within ~11%
            of rsqrt(deg), so one step leaves ~2% error on d for these chunks,
            which perturbs only the small P-correction term (~2% of the
            output) -> ~4e-4 relative, far inside the 2e-2 gate. T1 before PT:
            T2 (and thus every out chunk) depends on T1; PT[c] only gates the
            final out pair."""
            d_chain(c, c + 1, iters=iters)
            t1_mms(c, c + 1)
            pt_trans(c, c + 1)

        # G1 pacing: 2 per iter early, then 1
        # transpose all xx_nat groups up front (fills the PE gap before S,
        # and keeps the tensor engine p-state warm)
        for g in range(4):
            xpose_group(g)

        g1_sched = {0: [0, 1], 1: [2, 3], 2: [4, 5], 3: [6, 7], 4: [8, 9],
                    5: [10, 11], 6: [12], 7: [13], 8: [14], 9: [15]}

        # ---------- S phase: upper trapezoid + column sums ----------
        for i in range(NW):
            start_col = 128 * i
            parts = []
            c0 = start_col
            while c0 < W:
                w = min(1024, W - c0)
                parts.append((c0, w))
                c0 += w
            sig_tiles = []
            for pidx, (c0, w) in enumerate(parts):
                ps = psS.tile([128, 1024], f32, tag="s")
                o = 0
                while o < w:
                    n = min(512, w - o)
                    nc.tensor.matmul(
                        ps[:, o : o + n],
                        dqT[:, bass.ts(i, 128)],
                        xxT[:, c0 + o : c0 + o + n],
                        start=True,
                        stop=True,
                    )
                    o += n
                sg = sigp.tile([128, 1024], bf16, tag="sg")
                nc.scalar.activation(
                    sg[:, 0:w], ps[:, 0:w], AF.Sigmoid, accum_out=deg_parts[:, i, pidx : pidx + 1]
                )
                sig_tiles.append((sg, c0, w))
            # interleaved work first (keeps PE fed while ACT runs sigmoid)
            for gi in g1_sched.get(i, []):
                g1_tile(gi)
            if i in (4, 8, 12):
                g = (i - 4) // 4
                d_chain(4 * g, 4 * g + 4)
            if i in (5, 9, 13):
                g = (i - 5) // 4
                t1_mms(4 * g, 4 * g + 4)
                pt_trans(4 * g, 4 * g + 4)
            if i in (13, 14, 15):
                tail_chunk(i - 1)
            # column sums of off-diagonal chunks; per-block PSUM group + DVE accumulate
            if i < NW - 1:
                cs = psC.tile([128, NW], f32, tag="c")
                first = True
                for sg, c0, w in sig_tiles:
                    j0 = max(c0 // 128, i + 1)
                    for j in range(j0, (c0 + w) // 128):
                        nc.tensor.matmul(
                            cs[:, j : j + 1],
                            sg[:, 128 * j - c0 : 128 * (j + 1) - c0],
                            ones[:],
                            start=first,
                            stop=(j == NW - 1),
                        )
                        first = False
                nc.vector.tensor_tensor(
                    acc_cs[:, i + 1 : NW], acc_cs[:, i + 1 : NW], cs[:, i + 1 : NW], op=ALU.add
                )

        tail_chunk(15, iters=1)

        # T2 = (-cw) * T1 on ACT (Copy with per-partition scale)
        nc.scalar.activation(T2[:], pt1[:], AF.Copy, scale=ncw[:, 0:1])

        # ---------- out chunks: py = PT.T @ T2 (+G1); fused add on DVE/Pool,
        # ACT copy + PE ident-matmul. PSUM singles rotate across pools. ----------
        eng_pat = [("d", "a"), ("a", "d"), ("d", "a"), ("a", "d"),
                   ("d", "a"), ("a", "d"), ("d", "a"), ("a", "d")]
        dma_q = [nc.sync]
        for j in range(8):
            engs = eng_pat[j]
            if j % 4 in (0, 2):
                pyt = psS.tile([128, 1024], f32, tag="s")
                pys = [pyt[:, 0:512], pyt[:, 512:1024]]
            elif j % 4 == 1:
                pya0 = psA.tile([128, 512], f32, tag="a")
                pya1 = psA.tile([128, 512], f32, tag="a")
                pys = [pya0[:], pya1[:]]
            else:
                pyb = psB.tile([128, 512], f32, tag="b")
                pyc = psC.tile([128, 512], f32, tag="c")
                pys = [pyb[:], pyc[:]]
            st = outp.tile([128, 2, 512], bf16)
            for q in range(2):
                i = 2 * j + q
                use_act = engs[q] == "a"
                nc.tensor.matmul(pys[q], PT[:, bass.ts(i, 128)], T2[:],
                                 start=True, stop=not use_act)
                if use_act:
                    nc.tensor.matmul(pys[q], ident[:], G1[:, i, :], start=False, stop=True)
                    nc.scalar.activation(st[:, q, :], pys[q], AF.Copy)
                else:
                    nc.vector.tensor_tensor(st[:, q, :], pys[q], G1[:, i, :], op=ALU.add)
            if j == 0:
                for q in range(2):
                    dma_q[0].dma_start(
                        out_d[128 * q : 128 * (q + 1), :], st[:, q, :]
                    )
            else:
                dma_q[0].dma_start(
                    out_d[bass.ts(j, 256), :].rearrange("(q p) c -> p q c", p=128), st[:]
                )

    nc.compile()
    return nc


# ======================================================================
# Harness entry point: full inputs in, full output out.
# Shards batch B=8 across the 8 NeuronCores (pure data parallel).
# ======================================================================

_NC_CACHE = None


def _get_nc():
    global _NC_CACHE
    if _NC_CACHE is None:
        _NC_CACHE = build_nc()
    return _NC_CACHE


def _pack_kp(wT):
    """[C, M] weight-transpose -> [128, CQ*M] so partition p holds chunks k."""
    return np.ascontiguousarray(wT.reshape(CQ, 128, -1).transpose(1, 0, 2).reshape(128, -1))


def make_in_maps(x, fc_w, fc_b, avg_w, max_w, gcn_w):
    import ml_dtypes

    b16 = ml_dtypes.bfloat16
    x = np.asarray(x, dtype=np.float32)
    fc_w = np.asarray(fc_w, dtype=np.float32)
    fc_b = np.asarray(fc_b, dtype=np.float32)
    avg_w = np.asarray(avg_w, dtype=np.float32)
    max_w = np.asarray(max_w, dtype=np.float32)
    gcn_w = np.asarray(gcn_w, dtype=np.float32)
    shared = {
        "fcwT": _pack_kp(fc_w.T).astype(b16),
        "avgwT": _pack_kp(avg_w.T).astype(b16),
        "maxwT": _pack_kp(max_w.T).astype(b16),
        "fcb": np.ascontiguousarray(fc_b.reshape(1, M)).astype(b16),
        "gcn": _pack_kp(gcn_w).astype(b16),
        "ident": np.eye(128, dtype=np.float32).astype(b16),
    }
    return [
        {"xT": np.ascontiguousarray(x[b].T).astype(b16), **shared}
        for b in range(x.shape[0])
    ]


def kernel(x, fc_w, fc_b, avg_w, max_w, gcn_w):
    from concourse.bass_utils import run_bass_kernel_spmd

    nc = _get_nc()
    in_maps = make_in_maps(x, fc_w, fc_b, avg_w, max_w, gcn_w)
    res = run_bass_kernel_spmd(nc, in_maps, list(range(len(in_maps))))
    out = np.stack([np.asarray(res.results[b]["out"]) for b in range(len(in_maps))])
    return out.astype(np.float32)


# revision 47
# speedup vs baseline: 1.2859x; 1.0184x over previous
"""GNN message-passing kernel for TRN2, one batch element per NeuronCore.

Per-core math (x: [W=2048, C=512], weights replicated), all-bf16 dataflow:
  cw    = sigmoid(relu(mean_W(x)@avg_w.T) + relu(max_W(x)@max_w.T))   [M=128]
  xxT   = fc_w @ xT + b                         [M, W]
  dqT   = cw * xxT
  S     = dqT.T @ xxT  is symmetric -> upper trapezoid per row block;
          deg = trapezoid row sums (ACT sigmoid accum_out) + column sums of
          off-diagonal tiles (PE ones-matmuls on sigmoid(S) in bf16).
  d     = deg^-1/2   (DVE Newton rsqrt)
  G1    = x @ gcn_w                             [W, C]
  P     = d * xx  (nat layout, via PE transposes); PT = P^T
  T1    = P^T @ G1 ;  T2 = (-cw) * T1           [M, C]
  out_i = G1_i - PT_i.T @ (cw*T1)   (PE matmuls; G1-add fused into the
          PSUM->SBUF copy on DVE, or via PE ident-matmul for ACT copies)

Inputs are uploaded as bf16 (halves DMA), output is stored as bf16 and
converted to f32 on the host. Chunks 12-15 of the d/P/T1 pipeline run
per-chunk right behind the last S row blocks to shrink the tail.
"""

from contextlib import ExitStack

import numpy as np

import concourse.bass as bass
import concourse.tile as tile
from concourse import bacc, mybir

f32 = mybir.dt.float32
bf16 = mybir.dt.bfloat16
AF = mybir.ActivationFunctionType
ALU = mybir.AluOpType

W, C, M = 2048, 512, 128
CQ = C // 128      # 4 c-chunks
NW = W // 128      # 16 w-chunks
WS = W // 512      # 4 w-slices


def build_nc():
    nc = bacc.Bacc("TRN2", target_bir_lowering=False, debug=False, num_devices=8)

    xT_d = nc.dram_tensor("xT", [C, W], bf16, kind="ExternalInput").ap()
    fcwT_d = nc.dram_tensor("fcwT", [128, CQ * M], bf16, kind="ExternalInput").ap()
    avgwT_d = nc.dram_tensor("avgwT", [128, CQ * M], bf16, kind="ExternalInput").ap()
    maxwT_d = nc.dram_tensor("maxwT", [128, CQ * M], bf16, kind="ExternalInput").ap()
    fcb_d = nc.dram_tensor("fcb", [1, M], bf16, kind="ExternalInput").ap()
    gcn_d = nc.dram_tensor("gcn", [128, CQ * C], bf16, kind="ExternalInput").ap()
    ident_d = nc.dram_tensor("ident", [128, 128], bf16, kind="ExternalInput").ap()
    out_d = nc.dram_tensor("out", [W, C], bf16, kind="ExternalOutput").ap()

    with tile.TileContext(nc) as tc, ExitStack() as ctx:
        pool = ctx.enter_context(tc.tile_pool(name="sb", bufs=1))
        sigp = ctx.enter_context(tc.tile_pool(name="sigp", bufs=6))
        outp = ctx.enter_context(tc.tile_pool(name="outp", bufs=8))
        psS = ctx.enter_context(tc.tile_pool(name="psS", bufs=2, space="PSUM"))
        psA = ctx.enter_context(tc.tile_pool(name="psA", bufs=2, space="PSUM"))
        psB = ctx.enter_context(tc.tile_pool(name="psB", bufs=1, space="PSUM"))
        psC = ctx.enter_context(tc.tile_pool(name="psC", bufs=1, space="PSUM"))

        # ---------- persistent SBUF tensors ----------
        xT = pool.tile([128, CQ, W], bf16)          # x^T, c-chunk k on partitions
        fcwT = pool.tile([128, CQ, M], bf16)
        avgwT = pool.tile([128, CQ, M], bf16)
        maxwT = pool.tile([128, CQ, M], bf16)
        gcn = pool.tile([128, CQ, C], bf16)
        ident = pool.tile([128, 128], bf16)
        xxT = pool.tile([128, W], bf16)             # fc_w @ x^T + b   [M, W]
        dqT = pool.tile([128, W], bf16)             # cw * xxT
        xx_nat = pool.tile([128, NW, 128], bf16)    # xx w-chunks; scaled in place by d -> P
        PT = pool.tile([128, W], bf16)              # P^T  [M, W]
        G1 = pool.tile([128, NW, C], bf16)          # x @ gcn_w, w-chunk i on partitions
        T2 = pool.tile([128, C], bf16)
        dump = pool.tile([128, W], bf16)            # ACT stats dump target
        dump2 = pool.tile([128, W // 2], bf16)      # DVE TTR dump target
        mt1 = pool.tile([128, W // 2], bf16)        # max-tree scratch
        mt2 = pool.tile([128, W // 4], bf16)
        xsum_f = pool.tile([128, 8], f32)
        xmax_f = pool.tile([128, 4], f32)
        xsum_b = pool.tile([128, 4], bf16)
        xmax_b = pool.tile([128, 4], bf16)
        a_sb = pool.tile([128, 1], f32)
        m_sb = pool.tile([128, 1], f32)
        cw = pool.tile([128, 1], f32)
        ncw = pool.tile([128, 1], f32)
        ones = pool.tile([128, 1], bf16)
        ones_row = pool.tile([1, 512], bf16)
        fcb_row = pool.tile([1, 128], bf16)
        zeros1 = pool.tile([128, 1], f32)
        scr1 = pool.tile([128, 1], f32)
        deg_parts = pool.tile([128, NW, 2], f32)
        acc_cs = pool.tile([128, NW], f32)          # accumulated column sums (SBUF)
        deg = pool.tile([128, NW], f32)
        y_nr = pool.tile([128, NW], f32)            # rsqrt iterate -> d
        t_nr = pool.tile([128, NW], f32)
        u_nr = pool.tile([128, NW], f32)

        # Pin the ACT table set: make the first ACT instruction a Sigmoid.
        nc.gpsimd.memset(zeros1[:], 0.0)
        nc.scalar.activation(scr1[:], zeros1[:], AF.Sigmoid)
        nc.vector.memset(y_nr[:], 1.0 / 32.0)
        nc.vector.memset(ones[:], 1.0)
        nc.vector.memset(ones_row[:], 1.0)
        nc.vector.memset(deg_parts[:].rearrange("p a b -> p (a b)"), 0.0)
        nc.vector.memset(acc_cs[:], 0.0)

        # ---------- loads: fcwT, xT halves (k-major), small weights, gcn ----------
        nc.sync.dma_start(fcwT[:].rearrange("p k m -> p (k m)"), fcwT_d[:])
        for h in range(2 * CQ):
            k, p = h // 2, h % 2
            nc.sync.dma_start(xT[:, k, bass.ts(p, W // 2)], xT_d[bass.ts(k, 128), bass.ts(p, W // 2)])
        nc.sync.dma_start(avgwT[:].rearrange("p k m -> p (k m)"), avgwT_d[:])
        nc.sync.dma_start(maxwT[:].rearrange("p k m -> p (k m)"), maxwT_d[:])
        nc.sync.dma_start(fcb_row[:], fcb_d[:])
        nc.sync.dma_start(ident[:], ident_d[:])
        nc.sync.dma_start(gcn[:].rearrange("p k c -> p (k c)"), gcn_d[:])

        # ---------- stats during load ----------
        # ACT: chunk-0 sum as two halves (starts on first arrival), then
        # full-chunk sums for chunks 1-2. Chunk-3 sum lands on DVE as one
        # fused tensor_tensor_reduce.
        nc.scalar.activation(dump[:, 0 : W // 2], xT[:, 0, 0 : W // 2], AF.Copy, accum_out=xsum_f[:, 0:1])
        nc.scalar.activation(dump[:, 0 : W // 2], xT[:, 0, W // 2 : W], AF.Copy, accum_out=xsum_f[:, 6:7])
        for k in (1, 2):
            nc.scalar.activation(dump[:], xT[:, k, :], AF.Copy, accum_out=xsum_f[:, k : k + 1])

        # DVE: quarter-split max trees (first TT runs as soon as the first
        # half of a chunk lands)
        def max_tree(k):
            nc.vector.tensor_tensor(mt1[:, 0 : W // 4], xT[:, k, 0 : W // 4], xT[:, k, W // 4 : W // 2], op=ALU.max)
            nc.vector.tensor_tensor(mt1[:, W // 4 : W // 2], xT[:, k, W // 2 : 3 * W // 4], xT[:, k, 3 * W // 4 : W], op=ALU.max)
            nc.vector.tensor_tensor(mt2[:, 0 : W // 4], mt1[:, 0 : W // 4], mt1[:, W // 4 : W // 2], op=ALU.max)
            nc.vector.reduce_max(xmax_f[:, k : k + 1], mt2[:, 0 : W // 4], axis=mybir.AxisListType.X)

        for k in range(3):
            max_tree(k)
        # chunk-3 sum: STT add of the halves with accumulate (hw-safe
        # TensorScalarPtr form), then its max tree
        nc.vector.scalar_tensor_tensor(dump2[:], xT[:, 3, 0 : W // 2], 1.0, xT[:, 3, W // 2 : W],
                                       op0=ALU.mult, op1=ALU.add, accum_out=xsum_f[:, 4:5])
        max_tree(3)

        # ---------- xxT = fc_w @ xT + b: bias folded in as a rank-1 matmul
        # (fcb_row x ones_row) so the PSUM->SBUF copies are plain Copies,
        # split ACT (s0,s1) / DVE (s2,s3). gpsimd cannot read PSUM. ----------
        px0 = psA.tile([128, 512], f32, tag="a")
        px1 = psA.tile([128, 512], f32, tag="a")
        px23 = psS.tile([128, 1024], f32, tag="s")
        pxs = [px0[:], px1[:], px23[:, 0:512], px23[:, 512:1024]]
        for k in range(CQ):
            for s in (0, 1):
                nc.tensor.matmul(pxs[s], fcwT[:, k, :], xT[:, k, bass.ts(s, 512)], start=(k == 0), stop=False)
        for s in (0, 1):
            nc.tensor.matmul(pxs[s], fcb_row[:], ones_row[:], start=False, stop=True)
        for k in range(CQ):
            for s in (2, 3):
                nc.tensor.matmul(pxs[s], fcwT[:, k, :], xT[:, k, bass.ts(s, 512)], start=(k == 0), stop=False)
        for s in (2, 3):
            nc.tensor.matmul(pxs[s], fcb_row[:], ones_row[:], start=False, stop=True)
        # ---------- cw (emitted before the xxT copies so the relu/sigmoid
        # chain outranks them in ACT priority) ----------
        nc.vector.tensor_tensor(xsum_b[:, 0:1], xsum_f[:, 0:1], xsum_f[:, 6:7], op=ALU.add)
        for k in (1, 2):
            nc.vector.tensor_copy(xsum_b[:, k : k + 1], xsum_f[:, k : k + 1])
        nc.vector.tensor_copy(xsum_b[:, 3:4], xsum_f[:, 4:5])
        nc.vector.tensor_copy(xmax_b[:], xmax_f[:])
        pa = psC.tile([128, 16], f32, tag="c")
        for k in range(CQ):
            nc.tensor.matmul(pa[:, 0:1], avgwT[:, k, :], xsum_b[:, k : k + 1], start=(k == 0), stop=(k == CQ - 1))
        nc.scalar.activation(a_sb[:], pa[:, 0:1], AF.Relu, scale=1.0 / W)
        pm = psC.tile([128, 16], f32, tag="c")
        for k in range(CQ):
            nc.tensor.matmul(pm[:, 0:1], maxwT[:, k, :], xmax_b[:, k : k + 1], start=(k == 0), stop=(k == CQ - 1))
        nc.scalar.activation(m_sb[:], pm[:, 0:1], AF.Relu)
        nc.scalar.activation(cw[:], a_sb[:], AF.Sigmoid, bias=m_sb[:, 0:1])
        nc.vector.tensor_scalar_mul(ncw[:], cw[:], -1.0)

        for s in (0, 1):
            nc.scalar.activation(xxT[:, bass.ts(s, 512)], pxs[s], AF.Copy)
        nc.vector.tensor_copy(xxT[:, 1024:1536], pxs[2])
        # dqT = cw * xxT (bf16 4x TSP); s3 copy interleaved so dq0-2 can
        # slot in as soon as cw lands
        for s in (0, 1, 2):
            nc.vector.tensor_scalar_mul(dqT[:, bass.ts(s, 512)], xxT[:, bass.ts(s, 512)], cw[:, 0:1])
        nc.vector.tensor_copy(xxT[:, 1536:2048], pxs[3])
        nc.vector.tensor_scalar_mul(dqT[:, 1536:2048], xxT[:, 1536:2048], cw[:, 0:1])



        pt1 = None

        def g1_tile(i):
            pg = psA.tile([128, 512], f32, tag="a")
            for k in range(CQ):
                nc.tensor.matmul(pg[:], xT[:, k, bass.ts(i, 128)], gcn[:, k, :], start=(k == 0), stop=(k == CQ - 1))
            nc.vector.tensor_copy(G1[:, i, :], pg[:])

        def xpose_group(g):
            """Transpose 4 xxT blocks of group g into xx_nat (bf16 PSUM path).

            The PSUM->SBUF copy runs on Pool: keeping it off DVE avoids
            stealing DVE slots from the stats chain that gates cw."""
            pt = psA.tile([128, 512], bf16, tag="a")
            for q in range(4):
                blk = 4 * g + q
                nc.tensor.transpose(pt[:, bass.ts(q, 128)], xxT[:, bass.ts(blk, 128)], ident[:])
            nc.vector.tensor_copy(xx_nat[:, 4 * g : 4 * (g + 1), :].rearrange("p a b -> p (a b)"), pt[:])

        def d_chain(lo, hi, iters=3, eng=None):
            """rsqrt(deg) for chunks [lo, hi); scale xx_nat -> P in place.

            eng=gpsimd runs the SBUF-only math on the idle Pool engine
            (mid-phase groups); the latency-critical tail chunks use DVE.
            First Newton step from the constant seed y0=1/32 collapses to
            one affine op: y1 = y0*(1.5 - 0.5*deg*y0^2) = 3/64 - deg/65536."""
            e = eng if eng is not None else nc.vector
            sl = slice(lo, hi)
            e.tensor_tensor(deg[:, sl], deg_parts[:, sl, 0:1].rearrange("p a b -> p (a b)"),
                            deg_parts[:, sl, 1:2].rearrange("p a b -> p (a b)"), op=ALU.add)
            csl = slice(max(lo, 1), hi)  # column chunk 0 receives no colsums
            nc.vector.tensor_tensor(deg[:, csl], deg[:, csl], acc_cs[:, csl], op=ALU.add)
            e.tensor_scalar(y_nr[:, sl], deg[:, sl], -1.0 / 65536.0, 3.0 / 64.0, op0=ALU.mult, op1=ALU.add)
            for _ in range(iters - 1):
                e.tensor_tensor(t_nr[:, sl], y_nr[:, sl], y_nr[:, sl], op=ALU.mult)
                e.tensor_tensor(t_nr[:, sl], t_nr[:, sl], deg[:, sl], op=ALU.mult)
                e.tensor_scalar(u_nr[:, sl], t_nr[:, sl], -0.5, 1.5, op0=ALU.mult, op1=ALU.add)
                e.tensor_tensor(y_nr[:, sl], y_nr[:, sl], u_nr[:, sl], op=ALU.mult)
            for i in range(lo, hi):
                e.tensor_scalar_mul(xx_nat[:, i, :], xx_nat[:, i, :], y_nr[:, i : i + 1])

        def t1_mms(lo, hi):
            nonlocal pt1
            if pt1 is None:
                pt1 = psB.tile([128, 512], f32, tag="b")
            for i in range(lo, hi):
                nc.tensor.matmul(pt1[:], xx_nat[:, i, :], G1[:, i, :], start=(i == 0), stop=(i == NW - 1))

        def pt_trans(lo, hi):
            ptp = psA.tile([128, 512], bf16, tag="a")
            for i in range(lo, hi):
                nc.tensor.transpose(ptp[:, bass.ts(i - lo, 128)], xx_nat[:, i, :], ident[:])
            w = 128 * (hi - lo)
            nc.vector.tensor_copy(PT[:, 128 * lo : 128 * lo + w], ptp[:, 0:w])

        def tail_chunk(c, iters=1):
            """Per-chunk d -> scale -> T1 matmul -> PT transpose (chunks 12-15).

            One Newton iteration suffices here: the 1/32 seed is within ~11%
            of rsqrt(deg), so one step leaves ~2% error on d for these chunks,
            which perturbs only the small P-correction term (~2% of the
            output) -> ~4e-4 relative, far inside the 2e-2 gate. T1 before PT:
            T2 (and thus every out chunk) depends on T1; PT[c] only gates the
            final out pair."""
            d_chain(c, c + 1, iters=iters)
            t1_mms(c, c + 1)
            pt_trans(c, c + 1)

        # G1 pacing: 2 per iter early, then 1
        # transpose all xx_nat groups up front (fills the PE gap before S,
        # and keeps the tensor engine p-state warm)
        for g in range(4):
            xpose_group(g)

        g1_sched = {0: [0, 1], 1: [2, 3], 2: [4, 5], 3: [6, 7], 4: [8, 9],
                    5: [10], 6: [11], 7: [12], 8: [13], 9: [14], 10: [15]}

        # ---------- S phase: upper trapezoid + column sums ----------
        for i in range(NW):
            start_col = 128 * i
            parts = []
            c0 = start_col
            while c0 < W:
                w = min(1024, W - c0)
                parts.append((c0, w))
                c0 += w
            sig_tiles = []
            for pidx, (c0, w) in enumerate(parts):
                ps = psS.tile([128, 1024], f32, tag="s")
                o = 0
                while o < w:
                    n = min(512, w - o)
                    nc.tensor.matmul(
                        ps[:, o : o + n],
                        dqT[:, bass.ts(i, 128)],
                        xxT[:, c0 + o : c0 + o + n],
                        start=True,
                        stop=True,
                    )
                    o += n
                sg = sigp.tile([128, 1024], bf16, tag="sg")
                nc.scalar.activation(
                    sg[:, 0:w], ps[:, 0:w], AF.Sigmoid, accum_out=deg_parts[:, i, pidx : pidx + 1]
                )
                sig_tiles.append((sg, c0, w))
            # interleaved work first (keeps PE fed while ACT runs sigmoid)
            for gi in g1_sched.get(i, []):
                g1_tile(gi)
            if i in (4, 8, 12):
                g = (i - 4) // 4
                d_chain(4 * g, 4 * g + 4)
            if i in (5, 6, 9, 10, 13, 14):
                h2 = {5: 0, 6: 2, 9: 4, 10: 6, 13: 8, 14: 10}[i]
                t1_mms(h2, h2 + 2)
                pt_trans(h2, h2 + 2)
            if i in (13, 14, 15):
                tail_chunk(i - 1)
            # column sums of off-diagonal chunks; per-block PSUM group + DVE accumulate
            if i < NW - 1:
                cs = psC.tile([128, NW], f32, tag="c")
                first = True
                for sg, c0, w in sig_tiles:
                    j0 = max(c0 // 128, i + 1)
                    for j in range(j0, (c0 + w) // 128):
                        nc.tensor.matmul(
                            cs[:, j : j + 1],
                            sg[:, 128 * j - c0 : 128 * (j + 1) - c0],
                            ones[:],
                            start=first,
                            stop=(j == NW - 1),
                        )
                        first = False
                nc.vector.tensor_tensor(
                    acc_cs[:, i + 1 : NW], acc_cs[:, i + 1 : NW], cs[:, i + 1 : NW], op=ALU.add
                )

        tail_chunk(15, iters=1)

        # T2 = (-cw) * T1 on ACT (Copy with per-partition scale)
        nc.scalar.activation(T2[:], pt1[:], AF.Copy, scale=ncw[:, 0:1])

        # ---------- out chunks: py = PT.T @ T2 (+G1); fused add on DVE/Pool,
        # ACT copy + PE ident-matmul. PSUM singles rotate across pools. ----------
        eng_pat = [("d", "a"), ("a", "d"), ("d", "a"), ("a", "d"),
                   ("d", "a"), ("a", "d"), ("d", "a"), ("a", "d")]
        dma_q = [nc.sync]
        for j in range(8):
            engs = eng_pat[j]
            if j % 4 in (0, 2):
                pyt = psS.tile([128, 1024], f32, tag="s")
                pys = [pyt[:, 0:512], pyt[:, 512:1024]]
            elif j % 4 == 1:
                pya0 = psA.tile([128, 512], f32, tag="a")
                pya1 = psA.tile([128, 512], f32, tag="a")
                pys = [pya0[:], pya1[:]]
            else:
                pyb = psB.tile([128, 512], f32, tag="b")
                pyc = psC.tile([128, 512], f32, tag="c")
                pys = [pyb[:], pyc[:]]
            st = outp.tile([128, 2, 512], bf16)
            for q in range(2):
                i = 2 * j + q
                use_act = engs[q] == "a"
                nc.tensor.matmul(pys[q], PT[:, bass.ts(i, 128)], T2[:],
                                 start=True, stop=not use_act)
                if use_act:
                    nc.tensor.matmul(pys[q], ident[:], G1[:, i, :], start=False, stop=True)
                    nc.scalar.activation(st[:, q, :], pys[q], AF.Copy)
                else:
                    nc.vector.tensor_tensor(st[:, q, :], pys[q], G1[:, i, :], op=ALU.add)
            if j == 0:
                for q in range(2):
                    dma_q[0].dma_start(
                        out_d[128 * q : 128 * (q + 1), :], st[:, q, :]
                    )
            else:
                dma_q[0].dma_start(
                    out_d[bass.ts(j, 256), :].rearrange("(q p) c -> p q c", p=128), st[:]
                )

    nc.compile()
    return nc


# ======================================================================
# Harness entry point: full inputs in, full output out.
# Shards batch B=8 across the 8 NeuronCores (pure data parallel).
# ======================================================================

_NC_CACHE = None


def _get_nc():
    global _NC_CACHE
    if _NC_CACHE is None:
        _NC_CACHE = build_nc()
    return _NC_CACHE


def _pack_kp(wT):
    """[C, M] weight-transpose -> [128, CQ*M] so partition p holds chunks k."""
    return np.ascontiguousarray(wT.reshape(CQ, 128, -1).transpose(1, 0, 2).reshape(128, -1))


def make_in_maps(x, fc_w, fc_b, avg_w, max_w, gcn_w):
    import ml_dtypes

    b16 = ml_dtypes.bfloat16
    x = np.asarray(x, dtype=np.float32)
    fc_w = np.asarray(fc_w, dtype=np.float32)
    fc_b = np.asarray(fc_b, dtype=np.float32)
    avg_w = np.asarray(avg_w, dtype=np.float32)
    max_w = np.asarray(max_w, dtype=np.float32)
    gcn_w = np.asarray(gcn_w, dtype=np.float32)
    shared = {
        "fcwT": _pack_kp(fc_w.T).astype(b16),
        "avgwT": _pack_kp(avg_w.T).astype(b16),
        "maxwT": _pack_kp(max_w.T).astype(b16),
        "fcb": np.ascontiguousarray(fc_b.reshape(1, M)).astype(b16),
        "gcn": _pack_kp(gcn_w).astype(b16),
        "ident": np.eye(128, dtype=np.float32).astype(b16),
    }
    return [
        {"xT": np.ascontiguousarray(x[b].T).astype(b16), **shared}
        for b in range(x.shape[0])
    ]


def kernel(x, fc_w, fc_b, avg_w, max_w, gcn_w):
    from concourse.bass_utils import run_bass_kernel_spmd

    nc = _get_nc()
    in_maps = make_in_maps(x, fc_w, fc_b, avg_w, max_w, gcn_w)
    res = run_bass_kernel_spmd(nc, in_maps, list(range(len(in_maps))))
    out = np.stack([np.asarray(res.results[b]["out"]) for b in range(len(in_maps))])
    return out.astype(np.float32)


# revision 52
# speedup vs baseline: 1.2908x; 1.0038x over previous
"""GNN message-passing kernel for TRN2, one batch element per NeuronCore.

Per-core math (x: [W=2048, C=512], weights replicated), all-bf16 dataflow:
  cw    = sigmoid(relu(mean_W(x)@avg_w.T) + relu(max_W(x)@max_w.T))   [M=128]
  xxT   = fc_w @ xT + b                         [M, W]
  dqT   = cw * xxT
  S     = dqT.T @ xxT  is symmetric -> upper trapezoid per row block;
          deg = trapezoid row sums (ACT sigmoid accum_out) + column sums of
          off-diagonal tiles (PE ones-matmuls on sigmoid(S) in bf16).
  d     = deg^-1/2   (DVE Newton rsqrt)
  G1    = x @ gcn_w                             [W, C]
  P     = d * xx  (nat layout, via PE transposes); PT = P^T
  T1    = P^T @ G1 ;  T2 = (-cw) * T1           [M, C]
  out_i = G1_i - PT_i.T @ (cw*T1)   (PE matmuls; G1-add fused into the
          PSUM->SBUF copy on DVE, or via PE ident-matmul for ACT copies)

Inputs are uploaded as bf16 (halves DMA), output is stored as bf16 and
converted to f32 on the host. Chunks 12-15 of the d/P/T1 pipeline run
per-chunk right behind the last S row blocks to shrink the tail.
"""

from contextlib import ExitStack

import numpy as np

import concourse.bass as bass
import concourse.tile as tile
from concourse import bacc, mybir

f32 = mybir.dt.float32
bf16 = mybir.dt.bfloat16
AF = mybir.ActivationFunctionType
ALU = mybir.AluOpType

W, C, M = 2048, 512, 128
CQ = C // 128      # 4 c-chunks
NW = W // 128      # 16 w-chunks
WS = W // 512      # 4 w-slices


def build_nc():
    nc = bacc.Bacc("TRN2", target_bir_lowering=False, debug=False, num_devices=8)

    xT_d = nc.dram_tensor("xT", [C, W], bf16, kind="ExternalInput").ap()
    fcwT_d = nc.dram_tensor("fcwT", [128, CQ * M], bf16, kind="ExternalInput").ap()
    avgwT_d = nc.dram_tensor("avgwT", [128, CQ * M], bf16, kind="ExternalInput").ap()
    maxwT_d = nc.dram_tensor("maxwT", [128, CQ * M], bf16, kind="ExternalInput").ap()
    fcb_d = nc.dram_tensor("fcb", [1, M], bf16, kind="ExternalInput").ap()
    gcn_d = nc.dram_tensor("gcn", [128, CQ * C], bf16, kind="ExternalInput").ap()
    ident_d = nc.dram_tensor("ident", [128, 128], bf16, kind="ExternalInput").ap()
    out_d = nc.dram_tensor("out", [W, C], bf16, kind="ExternalOutput").ap()

    with tile.TileContext(nc) as tc, ExitStack() as ctx:
        pool = ctx.enter_context(tc.tile_pool(name="sb", bufs=1))
        sigp = ctx.enter_context(tc.tile_pool(name="sigp", bufs=6))
        outp = ctx.enter_context(tc.tile_pool(name="outp", bufs=8))
        psS = ctx.enter_context(tc.tile_pool(name="psS", bufs=2, space="PSUM"))
        psA = ctx.enter_context(tc.tile_pool(name="psA", bufs=2, space="PSUM"))
        psB = ctx.enter_context(tc.tile_pool(name="psB", bufs=1, space="PSUM"))
        psC = ctx.enter_context(tc.tile_pool(name="psC", bufs=1, space="PSUM"))

        # ---------- persistent SBUF tensors ----------
        xT = pool.tile([128, CQ, W], bf16)          # x^T, c-chunk k on partitions
        fcwT = pool.tile([128, CQ, M], bf16)
        avgwT = pool.tile([128, CQ, M], bf16)
        maxwT = pool.tile([128, CQ, M], bf16)
        gcn = pool.tile([128, CQ, C], bf16)
        ident = pool.tile([128, 128], bf16)
        xxT = pool.tile([128, W], bf16)             # fc_w @ x^T + b   [M, W]
        dqT = pool.tile([128, W], bf16)             # cw * xxT
        xx_nat = pool.tile([128, NW, 128], bf16)    # xx w-chunks; scaled in place by d -> P
        PT = pool.tile([128, W], bf16)              # P^T  [M, W]
        G1 = pool.tile([128, NW, C], bf16)          # x @ gcn_w, w-chunk i on partitions
        T2 = pool.tile([128, C], bf16)
        dump = pool.tile([128, W], bf16)            # ACT stats dump target
        dump2 = pool.tile([128, W // 2], bf16)      # DVE TTR dump target
        mt1 = pool.tile([128, W // 2], bf16)        # max-tree scratch
        mt2 = pool.tile([128, W // 4], bf16)
        xsum_f = pool.tile([128, 8], f32)
        xmax_f = pool.tile([128, 4], f32)
        xsum_b = pool.tile([128, 4], bf16)
        xmax_b = pool.tile([128, 4], bf16)
        a_sb = pool.tile([128, 1], f32)
        m_sb = pool.tile([128, 1], f32)
        cw = pool.tile([128, 1], f32)
        ncw = pool.tile([128, 1], f32)
        ones = pool.tile([128, 1], bf16)
        ones_row = pool.tile([1, 512], bf16)
        fcb_row = pool.tile([1, 128], bf16)
        zeros1 = pool.tile([128, 1], f32)
        scr1 = pool.tile([128, 1], f32)
        deg_parts = pool.tile([128, NW, 2], f32)
        acc_cs = pool.tile([128, NW], f32)          # accumulated column sums (SBUF)
        deg = pool.tile([128, NW], f32)
        y_nr = pool.tile([128, NW], f32)            # rsqrt iterate -> d
        t_nr = pool.tile([128, NW], f32)
        u_nr = pool.tile([128, NW], f32)

        # Pin the ACT table set: make the first ACT instruction a Sigmoid.
        nc.gpsimd.memset(zeros1[:], 0.0)
        nc.scalar.activation(scr1[:], zeros1[:], AF.Sigmoid)
        nc.vector.memset(y_nr[:], 1.0 / 32.0)
        nc.vector.memset(ones[:], 1.0)
        nc.vector.memset(ones_row[:], 1.0)
        nc.vector.memset(deg_parts[:].rearrange("p a b -> p (a b)"), 0.0)
        nc.vector.memset(acc_cs[:], 0.0)

        # ---------- loads: fcwT, xT halves (k-major), small weights, gcn ----------
        nc.sync.dma_start(fcwT[:].rearrange("p k m -> p (k m)"), fcwT_d[:])
        for h in range(2 * CQ):
            k, p = h // 2, h % 2
            nc.sync.dma_start(xT[:, k, bass.ts(p, W // 2)], xT_d[bass.ts(k, 128), bass.ts(p, W // 2)])
        nc.sync.dma_start(avgwT[:].rearrange("p k m -> p (k m)"), avgwT_d[:])
        nc.sync.dma_start(maxwT[:].rearrange("p k m -> p (k m)"), maxwT_d[:])
        nc.sync.dma_start(fcb_row[:], fcb_d[:])
        nc.sync.dma_start(ident[:], ident_d[:])
        nc.sync.dma_start(gcn[:].rearrange("p k c -> p (k c)"), gcn_d[:])

        # ---------- stats during load ----------
        # ACT: chunk-0 sum as two halves (starts on first arrival), then
        # full-chunk sums for chunks 1-2. Chunk-3 sum lands on DVE as one
        # fused tensor_tensor_reduce.
        nc.scalar.activation(dump[:, 0 : W // 2], xT[:, 0, 0 : W // 2], AF.Copy, accum_out=xsum_f[:, 0:1])
        nc.scalar.activation(dump[:, 0 : W // 2], xT[:, 0, W // 2 : W], AF.Copy, accum_out=xsum_f[:, 6:7])
        for k in (1, 2):
            nc.scalar.activation(dump[:], xT[:, k, :], AF.Copy, accum_out=xsum_f[:, k : k + 1])

        # DVE: quarter-split max trees (first TT runs as soon as the first
        # half of a chunk lands)
        def max_tree(k):
            nc.vector.tensor_tensor(mt1[:, 0 : W // 4], xT[:, k, 0 : W // 4], xT[:, k, W // 4 : W // 2], op=ALU.max)
            nc.vector.tensor_tensor(mt1[:, W // 4 : W // 2], xT[:, k, W // 2 : 3 * W // 4], xT[:, k, 3 * W // 4 : W], op=ALU.max)
            nc.vector.tensor_tensor(mt2[:, 0 : W // 4], mt1[:, 0 : W // 4], mt1[:, W // 4 : W // 2], op=ALU.max)
            nc.vector.reduce_max(xmax_f[:, k : k + 1], mt2[:, 0 : W // 4], axis=mybir.AxisListType.X)

        for k in range(3):
            max_tree(k)
        # chunk-3 sum: STT add of the halves with accumulate (hw-safe
        # TensorScalarPtr form), then its max tree
        nc.vector.scalar_tensor_tensor(dump2[:], xT[:, 3, 0 : W // 2], 1.0, xT[:, 3, W // 2 : W],
                                       op0=ALU.mult, op1=ALU.add, accum_out=xsum_f[:, 4:5])
        max_tree(3)

        # ---------- xxT = fc_w @ xT + b: bias folded in as a rank-1 matmul
        # (fcb_row x ones_row) so the PSUM->SBUF copies are plain Copies,
        # split ACT (s0,s1) / DVE (s2,s3). gpsimd cannot read PSUM. ----------
        px0 = psA.tile([128, 512], f32, tag="a")
        px1 = psA.tile([128, 512], f32, tag="a")
        px23 = psS.tile([128, 1024], f32, tag="s")
        pxs = [px0[:], px1[:], px23[:, 0:512], px23[:, 512:1024]]
        for k in range(CQ):
            for s in (0, 1):
                nc.tensor.matmul(pxs[s], fcwT[:, k, :], xT[:, k, bass.ts(s, 512)], start=(k == 0), stop=False)
        for s in (0, 1):
            nc.tensor.matmul(pxs[s], fcb_row[:], ones_row[:], start=False, stop=True)
        for k in range(CQ):
            for s in (2, 3):
                nc.tensor.matmul(pxs[s], fcwT[:, k, :], xT[:, k, bass.ts(s, 512)], start=(k == 0), stop=False)
        for s in (2, 3):
            nc.tensor.matmul(pxs[s], fcb_row[:], ones_row[:], start=False, stop=True)
        # ---------- cw (emitted before the xxT copies so the relu/sigmoid
        # chain outranks them in ACT priority) ----------
        nc.vector.tensor_tensor(xsum_b[:, 0:1], xsum_f[:, 0:1], xsum_f[:, 6:7], op=ALU.add)
        for k in (1, 2):
            nc.vector.tensor_copy(xsum_b[:, k : k + 1], xsum_f[:, k : k + 1])
        nc.vector.tensor_copy(xsum_b[:, 3:4], xsum_f[:, 4:5])
        nc.vector.tensor_copy(xmax_b[:], xmax_f[:])
        pa = psC.tile([128, 16], f32, tag="c")
        for k in range(CQ):
            nc.tensor.matmul(pa[:, 0:1], avgwT[:, k, :], xsum_b[:, k : k + 1], start=(k == 0), stop=(k == CQ - 1))
        nc.scalar.activation(a_sb[:], pa[:, 0:1], AF.Relu, scale=1.0 / W)
        pm = psC.tile([128, 16], f32, tag="c")
        for k in range(CQ):
            nc.tensor.matmul(pm[:, 0:1], maxwT[:, k, :], xmax_b[:, k : k + 1], start=(k == 0), stop=(k == CQ - 1))
        nc.scalar.activation(m_sb[:], pm[:, 0:1], AF.Relu)
        nc.scalar.activation(cw[:], a_sb[:], AF.Sigmoid, bias=m_sb[:, 0:1])
        nc.vector.tensor_scalar_mul(ncw[:], cw[:], -1.0)

        for s in (0, 1):
            nc.scalar.activation(xxT[:, bass.ts(s, 512)], pxs[s], AF.Copy)
        nc.vector.tensor_copy(xxT[:, 1024:1536], pxs[2])
        # dqT = cw * xxT (bf16 4x TSP); s3 copy interleaved so dq0-2 can
        # slot in as soon as cw lands
        for s in (0, 1, 2):
            nc.vector.tensor_scalar_mul(dqT[:, bass.ts(s, 512)], xxT[:, bass.ts(s, 512)], cw[:, 0:1])
        nc.vector.tensor_copy(xxT[:, 1536:2048], pxs[3])
        nc.vector.tensor_scalar_mul(dqT[:, 1536:2048], xxT[:, 1536:2048], cw[:, 0:1])



        pt1 = None

        def g1_tile(i):
            pg = psA.tile([128, 512], f32, tag="a")
            for k in range(CQ):
                nc.tensor.matmul(pg[:], xT[:, k, bass.ts(i, 128)], gcn[:, k, :], start=(k == 0), stop=(k == CQ - 1))
            nc.vector.tensor_copy(G1[:, i, :], pg[:])

        def xpose_group(g):
            """Transpose 4 xxT blocks of group g into xx_nat (bf16 PSUM path).

            The PSUM->SBUF copy runs on Pool: keeping it off DVE avoids
            stealing DVE slots from the stats chain that gates cw."""
            pt = psA.tile([128, 512], bf16, tag="a")
            for q in range(4):
                blk = 4 * g + q
                nc.tensor.transpose(pt[:, bass.ts(q, 128)], xxT[:, bass.ts(blk, 128)], ident[:])
            nc.vector.tensor_copy(xx_nat[:, 4 * g : 4 * (g + 1), :].rearrange("p a b -> p (a b)"), pt[:])

        def d_chain(lo, hi, iters=3, eng=None):
            """rsqrt(deg) for chunks [lo, hi); scale xx_nat -> P in place.

            eng=gpsimd runs the SBUF-only math on the idle Pool engine
            (mid-phase groups); the latency-critical tail chunks use DVE.
            First Newton step from the constant seed y0=1/32 collapses to
            one affine op: y1 = y0*(1.5 - 0.5*deg*y0^2) = 3/64 - deg/65536."""
            e = eng if eng is not None else nc.vector
            sl = slice(lo, hi)
            e.tensor_tensor(deg[:, sl], deg_parts[:, sl, 0:1].rearrange("p a b -> p (a b)"),
                            deg_parts[:, sl, 1:2].rearrange("p a b -> p (a b)"), op=ALU.add)
            csl = slice(max(lo, 1), hi)  # column chunk 0 receives no colsums
            nc.vector.tensor_tensor(deg[:, csl], deg[:, csl], acc_cs[:, csl], op=ALU.add)
            e.tensor_scalar(y_nr[:, sl], deg[:, sl], -1.0 / 65536.0, 3.0 / 64.0, op0=ALU.mult, op1=ALU.add)
            for _ in range(iters - 1):
                e.tensor_tensor(t_nr[:, sl], y_nr[:, sl], y_nr[:, sl], op=ALU.mult)
                e.tensor_tensor(t_nr[:, sl], t_nr[:, sl], deg[:, sl], op=ALU.mult)
                e.tensor_scalar(u_nr[:, sl], t_nr[:, sl], -0.5, 1.5, op0=ALU.mult, op1=ALU.add)
                e.tensor_tensor(y_nr[:, sl], y_nr[:, sl], u_nr[:, sl], op=ALU.mult)
            for i in range(lo, hi):
                e.tensor_scalar_mul(xx_nat[:, i, :], xx_nat[:, i, :], y_nr[:, i : i + 1])

        def t1_mms(lo, hi):
            nonlocal pt1
            if pt1 is None:
                pt1 = psB.tile([128, 512], f32, tag="b")
            for i in range(lo, hi):
                nc.tensor.matmul(pt1[:], xx_nat[:, i, :], G1[:, i, :], start=(i == 0), stop=(i == NW - 1))

        def pt_trans(lo, hi):
            ptp = psA.tile([128, 512], bf16, tag="a")
            for i in range(lo, hi):
                nc.tensor.transpose(ptp[:, bass.ts(i - lo, 128)], xx_nat[:, i, :], ident[:])
            w = 128 * (hi - lo)
            nc.vector.tensor_copy(PT[:, 128 * lo : 128 * lo + w], ptp[:, 0:w])

        def tail_chunk(c, iters=1):
            """Per-chunk d -> scale -> T1 matmul -> PT transpose (chunks 12-15).

            One Newton iteration suffices here: the 1/32 seed is within ~11%
            of rsqrt(deg), so one step leaves ~2% error on d for these chunks,
            which perturbs only the small P-correction term (~2% of the
            output) -> ~4e-4 relative, far inside the 2e-2 gate. T1 before PT:
            T2 (and thus every out chunk) depends on T1; PT[c] only gates the
            final out pair."""
            d_chain(c, c + 1, iters=iters)
            t1_mms(c, c + 1)
            pt_trans(c, c + 1)

        # G1 pacing: 2 per iter early, then 1
        # transpose all xx_nat groups up front (fills the PE gap before S,
        # and keeps the tensor engine p-state warm)
        for g in range(4):
            xpose_group(g)

        g1_sched = {0: [0, 1], 1: [2, 3], 2: [4, 5], 3: [6, 7], 4: [8, 9],
                    5: [10], 6: [11], 7: [12], 8: [13], 9: [14], 10: [15]}

        # ---------- S phase: upper trapezoid + column sums ----------
        for i in range(NW):
            start_col = 128 * i
            parts = []
            c0 = start_col
            while c0 < W:
                w = min(1024, W - c0)
                parts.append((c0, w))
                c0 += w
            sig_tiles = []
            for pidx, (c0, w) in enumerate(parts):
                ps = psS.tile([128, 1024], f32, tag="s")
                o = 0
                while o < w:
                    n = min(512, w - o)
                    nc.tensor.matmul(
                        ps[:, o : o + n],
                        dqT[:, bass.ts(i, 128)],
                        xxT[:, c0 + o : c0 + o + n],
                        start=True,
                        stop=True,
                    )
                    o += n
                sg = sigp.tile([128, 1024], bf16, tag="sg")
                nc.scalar.activation(
                    sg[:, 0:w], ps[:, 0:w], AF.Sigmoid, accum_out=deg_parts[:, i, pidx : pidx + 1]
                )
                sig_tiles.append((sg, c0, w))
            # interleaved work first (keeps PE fed while ACT runs sigmoid)
            for gi in g1_sched.get(i, []):
                g1_tile(gi)
            if i in (4, 8, 12):
                g = (i - 4) // 4
                d_chain(4 * g, 4 * g + 4)
            if i in (5, 6, 9, 10, 13, 14):
                h2 = {5: 0, 6: 2, 9: 4, 10: 6, 13: 8, 14: 10}[i]
                t1_mms(h2, h2 + 2)
                pt_trans(h2, h2 + 2)
            if i in (13, 14, 15):
                tail_chunk(i - 1)
            # column sums of off-diagonal chunks; per-block PSUM group + DVE accumulate
            if i < NW - 1:
                cs = psC.tile([128, NW], f32, tag="c")
                first = True
                for sg, c0, w in sig_tiles:
                    j0 = max(c0 // 128, i + 1)
                    for j in range(j0, (c0 + w) // 128):
                        nc.tensor.matmul(
                            cs[:, j : j + 1],
                            sg[:, 128 * j - c0 : 128 * (j + 1) - c0],
                            ones[:],
                            start=first,
                            stop=(j == NW - 1),
                        )
                        first = False
                nc.vector.tensor_tensor(
                    acc_cs[:, i + 1 : NW], acc_cs[:, i + 1 : NW], cs[:, i + 1 : NW], op=ALU.add
                )

        tail_chunk(15, iters=1)

        # T2 = (-cw) * T1 on ACT (Copy with per-partition scale)
        nc.scalar.activation(T2[:], pt1[:], AF.Copy, scale=ncw[:, 0:1])

        # ---------- out chunks: py = PT.T @ T2 (+G1); fused add on DVE/Pool,
        # ACT copy + PE ident-matmul. PSUM singles rotate across pools. ----------
        eng_pat = [("d", "a"), ("a", "d"), ("d", "a"), ("a", "d"),
                   ("d", "a"), ("a", "d"), ("d", "a"), ("a", "d")]
        dma_q = [nc.sync]
        for j in range(8):
            engs = eng_pat[j]
            if j % 4 in (0, 2):
                pyt = psS.tile([128, 1024], f32, tag="s")
                pys = [pyt[:, 0:512], pyt[:, 512:1024]]
            elif j % 4 == 1:
                pya0 = psA.tile([128, 512], f32, tag="a")
                pya1 = psA.tile([128, 512], f32, tag="a")
                pys = [pya0[:], pya1[:]]
            else:
                pyb = psB.tile([128, 512], f32, tag="b")
                pyc = psC.tile([128, 512], f32, tag="c")
                pys = [pyb[:], pyc[:]]
            st = outp.tile([128, 2, 512], bf16)
            for q in range(2):
                i = 2 * j + q
                use_act = engs[q] == "a"
                nc.tensor.matmul(pys[q], PT[:, bass.ts(i, 128)], T2[:],
                                 start=True, stop=not use_act)
                if use_act:
                    nc.tensor.matmul(pys[q], ident[:], G1[:, i, :], start=False, stop=True)
                    nc.scalar.activation(st[:, q, :], pys[q], AF.Copy)
                else:
                    nc.vector.tensor_tensor(st[:, q, :], pys[q], G1[:, i, :], op=ALU.add)
            if j in (0, 7):
                for q in range(2):
                    dma_q[0].dma_start(
                        out_d[256 * j + 128 * q : 256 * j + 128 * (q + 1), :], st[:, q, :]
                    )
            else:
                dma_q[0].dma_start(
                    out_d[bass.ts(j, 256), :].rearrange("(q p) c -> p q c", p=128), st[:]
                )

    nc.compile()
    return nc


# ======================================================================
# Harness entry point: full inputs in, full output out.
# Shards batch B=8 across the 8 NeuronCores (pure data parallel).
# ======================================================================

_NC_CACHE = None


def _get_nc():
    global _NC_CACHE
    if _NC_CACHE is None:
        _NC_CACHE = build_nc()
    return _NC_CACHE


def _pack_kp(wT):
    """[C, M] weight-transpose -> [128, CQ*M] so partition p holds chunks k."""
    return np.ascontiguousarray(wT.reshape(CQ, 128, -1).transpose(1, 0, 2).reshape(128, -1))


def make_in_maps(x, fc_w, fc_b, avg_w, max_w, gcn_w):
    import ml_dtypes

    b16 = ml_dtypes.bfloat16
    x = np.asarray(x, dtype=np.float32)
    fc_w = np.asarray(fc_w, dtype=np.float32)
    fc_b = np.asarray(fc_b, dtype=np.float32)
    avg_w = np.asarray(avg_w, dtype=np.float32)
    max_w = np.asarray(max_w, dtype=np.float32)
    gcn_w = np.asarray(gcn_w, dtype=np.float32)
    shared = {
        "fcwT": _pack_kp(fc_w.T).astype(b16),
        "avgwT": _pack_kp(avg_w.T).astype(b16),
        "maxwT": _pack_kp(max_w.T).astype(b16),
        "fcb": np.ascontiguousarray(fc_b.reshape(1, M)).astype(b16),
        "gcn": _pack_kp(gcn_w).astype(b16),
        "ident": np.eye(128, dtype=np.float32).astype(b16),
    }
    return [
        {"xT": np.ascontiguousarray(x[b].T).astype(b16), **shared}
        for b in range(x.shape[0])
    ]


def kernel(x, fc_w, fc_b, avg_w, max_w, gcn_w):
    from concourse.bass_utils import run_bass_kernel_spmd

    nc = _get_nc()
    in_maps = make_in_maps(x, fc_w, fc_b, avg_w, max_w, gcn_w)
    res = run_bass_kernel_spmd(nc, in_maps, list(range(len(in_maps))))
    out = np.stack([np.asarray(res.results[b]["out"]) for b in range(len(in_maps))])
    return out.astype(np.float32)


# revision 53
# speedup vs baseline: 1.3037x; 1.0100x over previous
"""GNN message-passing kernel for TRN2, one batch element per NeuronCore.

Per-core math (x: [W=2048, C=512], weights replicated), all-bf16 dataflow:
  cw    = sigmoid(relu(mean_W(x)@avg_w.T) + relu(max_W(x)@max_w.T))   [M=128]
  xxT   = fc_w @ xT + b                         [M, W]
  dqT   = cw * xxT
  S     = dqT.T @ xxT  is symmetric -> upper trapezoid per row block;
          deg = trapezoid row sums (ACT sigmoid accum_out) + column sums of
          off-diagonal tiles (PE ones-matmuls on sigmoid(S) in bf16).
  d     = deg^-1/2   (DVE Newton rsqrt)
  G1    = x @ gcn_w                             [W, C]
  P     = d * xx  (nat layout, via PE transposes); PT = P^T
  T1    = P^T @ G1 ;  T2 = (-cw) * T1           [M, C]
  out_i = G1_i - PT_i.T @ (cw*T1)   (PE matmuls; G1-add fused into the
          PSUM->SBUF copy on DVE, or via PE ident-matmul for ACT copies)

Inputs are uploaded as bf16 (halves DMA), output is stored as bf16 and
converted to f32 on the host. Chunks 12-15 of the d/P/T1 pipeline run
per-chunk right behind the last S row blocks to shrink the tail.
"""

from contextlib import ExitStack

import numpy as np

import concourse.bass as bass
import concourse.tile as tile
from concourse import bacc, mybir

f32 = mybir.dt.float32
bf16 = mybir.dt.bfloat16
AF = mybir.ActivationFunctionType
ALU = mybir.AluOpType

W, C, M = 2048, 512, 128
CQ = C // 128      # 4 c-chunks
NW = W // 128      # 16 w-chunks
WS = W // 512      # 4 w-slices


def build_nc():
    nc = bacc.Bacc("TRN2", target_bir_lowering=False, debug=False, num_devices=8)

    xT_d = nc.dram_tensor("xT", [C, W], bf16, kind="ExternalInput").ap()
    fcwT_d = nc.dram_tensor("fcwT", [128, CQ * M], bf16, kind="ExternalInput").ap()
    avgwT_d = nc.dram_tensor("avgwT", [128, CQ * M], bf16, kind="ExternalInput").ap()
    maxwT_d = nc.dram_tensor("maxwT", [128, CQ * M], bf16, kind="ExternalInput").ap()
    fcb_d = nc.dram_tensor("fcb", [1, M], bf16, kind="ExternalInput").ap()
    gcn_d = nc.dram_tensor("gcn", [128, CQ * C], bf16, kind="ExternalInput").ap()
    ident_d = nc.dram_tensor("ident", [128, 128], bf16, kind="ExternalInput").ap()
    out_d = nc.dram_tensor("out", [W, C], bf16, kind="ExternalOutput").ap()

    with tile.TileContext(nc) as tc, ExitStack() as ctx:
        pool = ctx.enter_context(tc.tile_pool(name="sb", bufs=1))
        sigp = ctx.enter_context(tc.tile_pool(name="sigp", bufs=6))
        outp = ctx.enter_context(tc.tile_pool(name="outp", bufs=8))
        psS = ctx.enter_context(tc.tile_pool(name="psS", bufs=2, space="PSUM"))
        psA = ctx.enter_context(tc.tile_pool(name="psA", bufs=2, space="PSUM"))
        psB = ctx.enter_context(tc.tile_pool(name="psB", bufs=1, space="PSUM"))
        psC = ctx.enter_context(tc.tile_pool(name="psC", bufs=1, space="PSUM"))

        # ---------- persistent SBUF tensors ----------
        xT = pool.tile([128, CQ, W], bf16)          # x^T, c-chunk k on partitions
        fcwT = pool.tile([128, CQ, M], bf16)
        avgwT = pool.tile([128, CQ, M], bf16)
        maxwT = pool.tile([128, CQ, M], bf16)
        gcn = pool.tile([128, CQ, C], bf16)
        ident = pool.tile([128, 128], bf16)
        xxT = pool.tile([128, W], bf16)             # fc_w @ x^T + b   [M, W]
        dqT = pool.tile([128, W], bf16)             # cw * xxT
        xx_nat = pool.tile([128, NW, 128], bf16)    # xx w-chunks; scaled in place by d -> P
        PT = pool.tile([128, W], bf16)              # P^T  [M, W]
        G1 = pool.tile([128, NW, C], bf16)          # x @ gcn_w, w-chunk i on partitions
        T2 = pool.tile([128, C], bf16)
        dump = pool.tile([128, W], bf16)            # ACT stats dump target
        dump2 = pool.tile([128, W // 2], bf16)      # DVE TTR dump target
        mt1 = pool.tile([128, W // 2], bf16)        # max-tree scratch
        mt2 = pool.tile([128, W // 4], bf16)
        xsum_f = pool.tile([128, 8], f32)
        xmax_f = pool.tile([128, 4], f32)
        xsum_b = pool.tile([128, 4], bf16)
        xmax_b = pool.tile([128, 4], bf16)
        a_sb = pool.tile([128, 1], f32)
        m_sb = pool.tile([128, 1], f32)
        cw = pool.tile([128, 1], f32)
        ncw = pool.tile([128, 1], f32)
        ones = pool.tile([128, 1], bf16)
        ones_row = pool.tile([1, 512], bf16)
        fcb_row = pool.tile([1, 128], bf16)
        zeros1 = pool.tile([128, 1], f32)
        scr1 = pool.tile([128, 1], f32)
        deg_parts = pool.tile([128, NW, 2], f32)
        acc_cs = pool.tile([128, NW], f32)          # accumulated column sums (SBUF)
        deg = pool.tile([128, NW], f32)
        y_nr = pool.tile([128, NW], f32)            # rsqrt iterate -> d
        t_nr = pool.tile([128, NW], f32)
        u_nr = pool.tile([128, NW], f32)

        # Pin the ACT table set: make the first ACT instruction a Sigmoid.
        nc.gpsimd.memset(zeros1[:], 0.0)
        nc.scalar.activation(scr1[:], zeros1[:], AF.Sigmoid)
        nc.vector.memset(y_nr[:], 1.0 / 32.0)
        nc.vector.memset(ones[:], 1.0)
        nc.vector.memset(ones_row[:], 1.0)
        nc.vector.memset(deg_parts[:].rearrange("p a b -> p (a b)"), 0.0)
        nc.vector.memset(acc_cs[:], 0.0)

        # ---------- loads: fcwT, xT halves (k-major), small weights, gcn ----------
        nc.sync.dma_start(fcwT[:].rearrange("p k m -> p (k m)"), fcwT_d[:])
        for h in range(2 * CQ):
            k, p = h // 2, h % 2
            nc.sync.dma_start(xT[:, k, bass.ts(p, W // 2)], xT_d[bass.ts(k, 128), bass.ts(p, W // 2)])
        nc.sync.dma_start(avgwT[:].rearrange("p k m -> p (k m)"), avgwT_d[:])
        nc.sync.dma_start(maxwT[:].rearrange("p k m -> p (k m)"), maxwT_d[:])
        nc.sync.dma_start(fcb_row[:], fcb_d[:])
        nc.sync.dma_start(ident[:], ident_d[:])
        nc.sync.dma_start(gcn[:].rearrange("p k c -> p (k c)"), gcn_d[:])

        # ---------- stats during load ----------
        # ACT: chunk-0 sum as two halves (starts on first arrival), then
        # full-chunk sums for chunks 1-2. Chunk-3 sum lands on DVE as one
        # fused tensor_tensor_reduce.
        nc.scalar.activation(dump[:, 0 : W // 2], xT[:, 0, 0 : W // 2], AF.Copy, accum_out=xsum_f[:, 0:1])
        nc.scalar.activation(dump[:, 0 : W // 2], xT[:, 0, W // 2 : W], AF.Copy, accum_out=xsum_f[:, 6:7])
        for k in (1, 2):
            nc.scalar.activation(dump[:], xT[:, k, :], AF.Copy, accum_out=xsum_f[:, k : k + 1])

        # DVE: quarter-split max trees (first TT runs as soon as the first
        # half of a chunk lands)
        def max_tree(k):
            nc.vector.tensor_tensor(mt1[:, 0 : W // 4], xT[:, k, 0 : W // 4], xT[:, k, W // 4 : W // 2], op=ALU.max)
            nc.vector.tensor_tensor(mt1[:, W // 4 : W // 2], xT[:, k, W // 2 : 3 * W // 4], xT[:, k, 3 * W // 4 : W], op=ALU.max)
            nc.vector.tensor_tensor(mt2[:, 0 : W // 4], mt1[:, 0 : W // 4], mt1[:, W // 4 : W // 2], op=ALU.max)
            nc.vector.reduce_max(xmax_f[:, k : k + 1], mt2[:, 0 : W // 4], axis=mybir.AxisListType.X)

        for k in range(3):
            max_tree(k)
        # chunk-3 sum: STT add of the halves with accumulate (hw-safe
        # TensorScalarPtr form), then its max tree
        nc.vector.scalar_tensor_tensor(dump2[:], xT[:, 3, 0 : W // 2], 1.0, xT[:, 3, W // 2 : W],
                                       op0=ALU.mult, op1=ALU.add, accum_out=xsum_f[:, 4:5])
        max_tree(3)

        # ---------- xxT = fc_w @ xT + b: bias folded in as a rank-1 matmul
        # (fcb_row x ones_row) so the PSUM->SBUF copies are plain Copies,
        # split ACT (s0,s1) / DVE (s2,s3). gpsimd cannot read PSUM. ----------
        px0 = psA.tile([128, 512], f32, tag="a")
        px1 = psA.tile([128, 512], f32, tag="a")
        px23 = psS.tile([128, 1024], f32, tag="s")
        pxs = [px0[:], px1[:], px23[:, 0:512], px23[:, 512:1024]]
        for k in range(CQ):
            for s in (0, 1):
                nc.tensor.matmul(pxs[s], fcwT[:, k, :], xT[:, k, bass.ts(s, 512)], start=(k == 0), stop=False)
        for s in (0, 1):
            nc.tensor.matmul(pxs[s], fcb_row[:], ones_row[:], start=False, stop=True)
        for k in range(CQ):
            for s in (2, 3):
                nc.tensor.matmul(pxs[s], fcwT[:, k, :], xT[:, k, bass.ts(s, 512)], start=(k == 0), stop=False)
        for s in (2, 3):
            nc.tensor.matmul(pxs[s], fcb_row[:], ones_row[:], start=False, stop=True)
        # ---------- cw (emitted before the xxT copies so the relu/sigmoid
        # chain outranks them in ACT priority) ----------
        nc.vector.tensor_tensor(xsum_b[:, 0:1], xsum_f[:, 0:1], xsum_f[:, 6:7], op=ALU.add)
        for k in (1, 2):
            nc.vector.tensor_copy(xsum_b[:, k : k + 1], xsum_f[:, k : k + 1])
        nc.vector.tensor_copy(xsum_b[:, 3:4], xsum_f[:, 4:5])
        nc.vector.tensor_copy(xmax_b[:], xmax_f[:])
        pa = psC.tile([128, 16], f32, tag="c")
        for k in range(CQ):
            nc.tensor.matmul(pa[:, 0:1], avgwT[:, k, :], xsum_b[:, k : k + 1], start=(k == 0), stop=(k == CQ - 1))
        nc.scalar.activation(a_sb[:], pa[:, 0:1], AF.Relu, scale=1.0 / W)
        pm = psC.tile([128, 16], f32, tag="c")
        for k in range(CQ):
            nc.tensor.matmul(pm[:, 0:1], maxwT[:, k, :], xmax_b[:, k : k + 1], start=(k == 0), stop=(k == CQ - 1))
        nc.scalar.activation(m_sb[:], pm[:, 0:1], AF.Relu)
        nc.scalar.activation(cw[:], a_sb[:], AF.Sigmoid, bias=m_sb[:, 0:1])
        nc.vector.tensor_scalar_mul(ncw[:], cw[:], -1.0)

        for s in (0, 1):
            nc.scalar.activation(xxT[:, bass.ts(s, 512)], pxs[s], AF.Copy)
        nc.scalar.activation(xxT[:, 1024:1536], pxs[2], AF.Copy)
        # dqT = cw * xxT (bf16 4x TSP); s3 copy interleaved so dq0-2 can
        # slot in as soon as cw lands
        for s in (0, 1, 2):
            nc.vector.tensor_scalar_mul(dqT[:, bass.ts(s, 512)], xxT[:, bass.ts(s, 512)], cw[:, 0:1])
        nc.vector.tensor_copy(xxT[:, 1536:2048], pxs[3])
        nc.vector.tensor_scalar_mul(dqT[:, 1536:2048], xxT[:, 1536:2048], cw[:, 0:1])



        pt1 = None

        def g1_tile(i):
            pg = psA.tile([128, 512], f32, tag="a")
            for k in range(CQ):
                nc.tensor.matmul(pg[:], xT[:, k, bass.ts(i, 128)], gcn[:, k, :], start=(k == 0), stop=(k == CQ - 1))
            nc.vector.tensor_copy(G1[:, i, :], pg[:])

        def xpose_group(g):
            """Transpose 4 xxT blocks of group g into xx_nat (bf16 PSUM path).

            The PSUM->SBUF copy runs on Pool: keeping it off DVE avoids
            stealing DVE slots from the stats chain that gates cw."""
            pt = psA.tile([128, 512], bf16, tag="a")
            for q in range(4):
                blk = 4 * g + q
                nc.tensor.transpose(pt[:, bass.ts(q, 128)], xxT[:, bass.ts(blk, 128)], ident[:])
            nc.scalar.activation(xx_nat[:, 4 * g : 4 * (g + 1), :].rearrange("p a b -> p (a b)"), pt[:], AF.Copy)

        def d_chain(lo, hi, iters=3, eng=None):
            """rsqrt(deg) for chunks [lo, hi); scale xx_nat -> P in place.

            eng=gpsimd runs the SBUF-only math on the idle Pool engine
            (mid-phase groups); the latency-critical tail chunks use DVE.
            First Newton step from the constant seed y0=1/32 collapses to
            one affine op: y1 = y0*(1.5 - 0.5*deg*y0^2) = 3/64 - deg/65536."""
            e = eng if eng is not None else nc.vector
            sl = slice(lo, hi)
            e.tensor_tensor(deg[:, sl], deg_parts[:, sl, 0:1].rearrange("p a b -> p (a b)"),
                            deg_parts[:, sl, 1:2].rearrange("p a b -> p (a b)"), op=ALU.add)
            csl = slice(max(lo, 1), hi)  # column chunk 0 receives no colsums
            nc.vector.tensor_tensor(deg[:, csl], deg[:, csl], acc_cs[:, csl], op=ALU.add)
            e.tensor_scalar(y_nr[:, sl], deg[:, sl], -1.0 / 65536.0, 3.0 / 64.0, op0=ALU.mult, op1=ALU.add)
            for _ in range(iters - 1):
                e.tensor_tensor(t_nr[:, sl], y_nr[:, sl], y_nr[:, sl], op=ALU.mult)
                e.tensor_tensor(t_nr[:, sl], t_nr[:, sl], deg[:, sl], op=ALU.mult)
                e.tensor_scalar(u_nr[:, sl], t_nr[:, sl], -0.5, 1.5, op0=ALU.mult, op1=ALU.add)
                e.tensor_tensor(y_nr[:, sl], y_nr[:, sl], u_nr[:, sl], op=ALU.mult)
            for i in range(lo, hi):
                e.tensor_scalar_mul(xx_nat[:, i, :], xx_nat[:, i, :], y_nr[:, i : i + 1])

        def t1_mms(lo, hi):
            nonlocal pt1
            if pt1 is None:
                pt1 = psB.tile([128, 512], f32, tag="b")
            for i in range(lo, hi):
                nc.tensor.matmul(pt1[:], xx_nat[:, i, :], G1[:, i, :], start=(i == 0), stop=(i == NW - 1))

        def pt_trans(lo, hi):
            ptp = psA.tile([128, 512], bf16, tag="a")
            for i in range(lo, hi):
                nc.tensor.transpose(ptp[:, bass.ts(i - lo, 128)], xx_nat[:, i, :], ident[:])
            w = 128 * (hi - lo)
            nc.vector.tensor_copy(PT[:, 128 * lo : 128 * lo + w], ptp[:, 0:w])

        def tail_chunk(c, iters=1):
            """Per-chunk d -> scale -> T1 matmul -> PT transpose (chunks 12-15).

            One Newton iteration suffices here: the 1/32 seed is within ~11%
            of rsqrt(deg), so one step leaves ~2% error on d for these chunks,
            which perturbs only the small P-correction term (~2% of the
            output) -> ~4e-4 relative, far inside the 2e-2 gate. T1 before PT:
            T2 (and thus every out chunk) depends on T1; PT[c] only gates the
            final out pair."""
            d_chain(c, c + 1, iters=iters)
            t1_mms(c, c + 1)
            pt_trans(c, c + 1)

        # G1 pacing: 2 per iter early, then 1
        # transpose all xx_nat groups up front (fills the PE gap before S,
        # and keeps the tensor engine p-state warm)
        for g in range(4):
            xpose_group(g)

        g1_sched = {0: [0, 1], 1: [2, 3], 2: [4, 5], 3: [6, 7], 4: [8, 9],
                    5: [10], 6: [11], 7: [12], 8: [13], 9: [14], 10: [15]}

        # ---------- S phase: upper trapezoid + column sums ----------
        for i in range(NW):
            start_col = 128 * i
            parts = []
            c0 = start_col
            while c0 < W:
                w = min(1024, W - c0)
                parts.append((c0, w))
                c0 += w
            sig_tiles = []
            for pidx, (c0, w) in enumerate(parts):
                ps = psS.tile([128, 1024], f32, tag="s")
                o = 0
                while o < w:
                    n = min(512, w - o)
                    nc.tensor.matmul(
                        ps[:, o : o + n],
                        dqT[:, bass.ts(i, 128)],
                        xxT[:, c0 + o : c0 + o + n],
                        start=True,
                        stop=True,
                    )
                    o += n
                sg = sigp.tile([128, 1024], bf16, tag="sg")
                nc.scalar.activation(
                    sg[:, 0:w], ps[:, 0:w], AF.Sigmoid, accum_out=deg_parts[:, i, pidx : pidx + 1]
                )
                sig_tiles.append((sg, c0, w))
            # interleaved work first (keeps PE fed while ACT runs sigmoid)
            for gi in g1_sched.get(i, []):
                g1_tile(gi)
            if i in (4, 8, 12):
                g = (i - 4) // 4
                d_chain(4 * g, 4 * g + 4)
            if i in (5, 6, 9, 10, 13, 14):
                h2 = {5: 0, 6: 2, 9: 4, 10: 6, 13: 8, 14: 10}[i]
                t1_mms(h2, h2 + 2)
                pt_trans(h2, h2 + 2)
            if i in (13, 14, 15):
                tail_chunk(i - 1)
            # column sums of off-diagonal chunks; per-block PSUM group + DVE accumulate
            if i < NW - 1:
                cs = psC.tile([128, NW], f32, tag="c")
                first = True
                for sg, c0, w in sig_tiles:
                    j0 = max(c0 // 128, i + 1)
                    for j in range(j0, (c0 + w) // 128):
                        nc.tensor.matmul(
                            cs[:, j : j + 1],
                            sg[:, 128 * j - c0 : 128 * (j + 1) - c0],
                            ones[:],
                            start=first,
                            stop=(j == NW - 1),
                        )
                        first = False
                nc.vector.tensor_tensor(
                    acc_cs[:, i + 1 : NW], acc_cs[:, i + 1 : NW], cs[:, i + 1 : NW], op=ALU.add
                )

        tail_chunk(15, iters=1)

        # T2 = (-cw) * T1 on ACT (Copy with per-partition scale)
        nc.scalar.activation(T2[:], pt1[:], AF.Copy, scale=ncw[:, 0:1])

        # ---------- out chunks: py = PT.T @ T2 (+G1); fused add on DVE/Pool,
        # ACT copy + PE ident-matmul. PSUM singles rotate across pools. ----------
        eng_pat = [("d", "a"), ("a", "d"), ("d", "a"), ("a", "d"),
                   ("d", "a"), ("a", "d"), ("d", "a"), ("a", "d")]
        dma_q = [nc.sync]
        for j in range(8):
            engs = eng_pat[j]
            if j % 4 in (0, 2):
                pyt = psS.tile([128, 1024], f32, tag="s")
                pys = [pyt[:, 0:512], pyt[:, 512:1024]]
            elif j % 4 == 1:
                pya0 = psA.tile([128, 512], f32, tag="a")
                pya1 = psA.tile([128, 512], f32, tag="a")
                pys = [pya0[:], pya1[:]]
            else:
                pyb = psB.tile([128, 512], f32, tag="b")
                pyc = psC.tile([128, 512], f32, tag="c")
                pys = [pyb[:], pyc[:]]
            st = outp.tile([128, 2, 512], bf16)
            for q in range(2):
                i = 2 * j + q
                use_act = engs[q] == "a"
                nc.tensor.matmul(pys[q], PT[:, bass.ts(i, 128)], T2[:],
                                 start=True, stop=not use_act)
                if use_act:
                    nc.tensor.matmul(pys[q], ident[:], G1[:, i, :], start=False, stop=True)
                    nc.scalar.activation(st[:, q, :], pys[q], AF.Copy)
                else:
                    nc.vector.tensor_tensor(st[:, q, :], pys[q], G1[:, i, :], op=ALU.add)
            if j in (0, 7):
                for q in range(2):
                    dma_q[0].dma_start(
                        out_d[256 * j + 128 * q : 256 * j + 128 * (q + 1), :], st[:, q, :]
                    )
            else:
                dma_q[0].dma_start(
                    out_d[bass.ts(j, 256), :].rearrange("(q p) c -> p q c", p=128), st[:]
                )

    nc.compile()
    return nc


# ======================================================================
# Harness entry point: full inputs in, full output out.
# Shards batch B=8 across the 8 NeuronCores (pure data parallel).
# ======================================================================

_NC_CACHE = None


def _get_nc():
    global _NC_CACHE
    if _NC_CACHE is None:
        _NC_CACHE = build_nc()
    return _NC_CACHE


def _pack_kp(wT):
    """[C, M] weight-transpose -> [128, CQ*M] so partition p holds chunks k."""
    return np.ascontiguousarray(wT.reshape(CQ, 128, -1).transpose(1, 0, 2).reshape(128, -1))


def make_in_maps(x, fc_w, fc_b, avg_w, max_w, gcn_w):
    import ml_dtypes

    b16 = ml_dtypes.bfloat16
    x = np.asarray(x, dtype=np.float32)
    fc_w = np.asarray(fc_w, dtype=np.float32)
    fc_b = np.asarray(fc_b, dtype=np.float32)
    avg_w = np.asarray(avg_w, dtype=np.float32)
    max_w = np.asarray(max_w, dtype=np.float32)
    gcn_w = np.asarray(gcn_w, dtype=np.float32)
    shared = {
        "fcwT": _pack_kp(fc_w.T).astype(b16),
        "avgwT": _pack_kp(avg_w.T).astype(b16),
        "maxwT": _pack_kp(max_w.T).astype(b16),
        "fcb": np.ascontiguousarray(fc_b.reshape(1, M)).astype(b16),
        "gcn": _pack_kp(gcn_w).astype(b16),
        "ident": np.eye(128, dtype=np.float32).astype(b16),
    }
    return [
        {"xT": np.ascontiguousarray(x[b].T).astype(b16), **shared}
        for b in range(x.shape[0])
    ]


def kernel(x, fc_w, fc_b, avg_w, max_w, gcn_w):
    from concourse.bass_utils import run_bass_kernel_spmd

    nc = _get_nc()
    in_maps = make_in_maps(x, fc_w, fc_b, avg_w, max_w, gcn_w)
    res = run_bass_kernel_spmd(nc, in_maps, list(range(len(in_maps))))
    out = np.stack([np.asarray(res.results[b]["out"]) for b in range(len(in_maps))])
    return out.astype(np.float32)
